# revision 1
# baseline (speedup 1.0000x reference)
"""Bass/Trainium2 kernel for the (dead-attention) GAT reference.

Effective math (see reference):
    h1  = x @ W1f                 W1f = W1.transpose(1,0,2).reshape(256,128)
    hp1 = elu(adj @ h1)
    h2  = hp1 @ W2f               W2f = W2.transpose(1,0,2).reshape(128,128)
    hp2 = elu(adj @ h2)
    y   = elu(hp2 @ Wout + bout)
    out = log_softmax(y, axis=1)

Distribution: adj is sharded row-wise across 8 cores (2048 rows each),
uploaded pre-transposed + fp16, with the CONTRACTION rows rotated per
core so each core's own nodes come first.  h1 is computed REPLICATED on
every core (x is tiny), so layer 1 needs no collective.  h2 is
exchanged with one fp16 AllGather in feature-major layout; thanks to
the rotation each core starts layer 2 on its own h2 shard (local, no
collective wait) while the AllGather flies, then pulls the other 7
blocks with partition-id-indexed dynamic DMAs and transposes them back
to node-major lhsT tiles on the PE.  Each core streams its 67 MB adj
shard from HBM through the PE array twice:
    hpT[128 feat, 2048 rows] = sum_k h[kblk 128 rows].T-stationary @ adjT[kblk]
fp32 accumulation in PSUM; fp16 on the streamed matmuls (max elementwise
rel err vs the fp32 reference ~6e-4).
"""

import sys

import numpy as np

sys.path.insert(0, "/opt/trn_rl_repo")

N = 16384  # nodes
F = 256  # input features
D = 128  # hidden width (nheads*nhid)
C = 32  # classes
NCORES = 8
S = N // NCORES  # rows per core

_nc_cache = {}


def build_gat_nc(n_total=N, ncores=NCORES, enable_asserts=False, adj_bufs=7, kg=4):
    """Build the SPMD Bass program (one program, runs on all cores)."""
    from concourse import bacc, bass, masks, mybir, tile

    s = n_total // ncores  # shard rows per core
    kb = n_total // 128  # contraction blocks for the big matmul
    kb8 = kb // 8  # x chunk groups
    rc = s // 128  # 128-row chunks in this core's shard
    f32 = mybir.dt.float32
    f16 = mybir.dt.float16
    AF = mybir.ActivationFunctionType
    OP = mybir.AluOpType
    # n-chunks of the big-matmul output (<=512 fp32 per PSUM bank)
    nw = [min(512, s - i) for i in range(0, s, 512)]
    no = [i for i in range(0, s, 512)]

    nc = bacc.Bacc(
        "TRN2",
        target_bir_lowering=False,
        debug=False,
        enable_asserts=enable_asserts,
        num_devices=ncores,
    )

    adjt = nc.dram_tensor("adjt", [n_total, s], f16, kind="ExternalInput")
    xc = nc.dram_tensor("xc", [kb8 * 128, 8 * F], f16, kind="ExternalInput")
    w1 = nc.dram_tensor("w1", [F, D], f16, kind="ExternalInput")
    w2 = nc.dram_tensor("w2", [D, D], f16, kind="ExternalInput")
    wout = nc.dram_tensor("wout", [D, C], f32, kind="ExternalInput")
    bb = nc.dram_tensor("bb", [128, C], f32, kind="ExternalInput")
    # hoff[0, g] = ((rank + 1 + g) % ncores) * 128: gather-block row offsets
    hoff = nc.dram_tensor("hoff", [1, 8], mybir.dt.uint32, kind="ExternalInput")
    out = nc.dram_tensor("out", [s, C], f32, kind="ExternalOutput")

    rg = [list(range(ncores))]

    with tile.TileContext(nc) as tc:
        with (
            tc.tile_pool(name="dram", bufs=1, space="DRAM") as dram,
            tc.tile_pool(name="const", bufs=1) as const,
            tc.tile_pool(name="hfull", bufs=1) as hpool,
            tc.tile_pool(name="adjs", bufs=adj_bufs) as apool,
            tc.tile_pool(name="hblkp", bufs=2) as hblkp,
            tc.tile_pool(name="xcp", bufs=2) as xcpool,
            tc.tile_pool(name="xe", bufs=2) as xepool,
            tc.tile_pool(name="hsb", bufs=2) as hsbpool,
            tc.tile_pool(name="tmp", bufs=1) as tmp,
            tc.tile_pool(name="outp", bufs=2) as outp,
            tc.tile_pool(name="stat", bufs=1) as stat,
            tc.tile_pool(name="psb", bufs=4, space="PSUM") as psb,
            tc.tile_pool(name="pss", bufs=2, space="PSUM") as pss,
            tc.tile_pool(name="psy", bufs=2, space="PSUM") as psy,
        ):
            # two HWDGE rings (sync/scalar) alternate the big adj stream;
            # constants + tiny stores go to the SWDGE path (gpsimd)
            ringA, ringB, ringC = nc.sync, nc.scalar, nc.gpsimd

            # --- replicated constants (SWDGE so rings start streaming) ---
            w1s = const.tile([128, 2, D], f16, tag="w1s")
            ringC.dma_start(w1s[:], w1.ap().rearrange("(a p) m -> p a m", p=128))
            w2s = const.tile([128, D], f16, tag="w2s")
            ringC.dma_start(w2s[:], w2.ap())
            wouts = const.tile([128, C], f32, tag="wouts")
            ringC.dma_start(wouts[:], wout.ap())
            bbs = const.tile([128, C], f32, tag="bbs")
            ringC.dma_start(bbs[:], bb.ap())
            hoffs = const.tile([1, 8], mybir.dt.uint32, tag="hoffs")
            ringC.dma_start(hoffs[:], hoff.ap())
            ident = const.tile([128, 128], f16, tag="ident")
            masks.make_identity(nc, ident[:])

            # --- DRAM bounce buffers for the collective (feature-major) ---
            h2b = dram.tile([128, s], f16, tag="h2b")
            h2f = dram.tile([128 * ncores, s], f16, tag="h2f", addr_space="Shared")

            def big_layer(hs):
                # hpT[128 feat, s rows] += h[kblk].T-stationary @ adjT[kblk]
                ps = [
                    psb.tile([128, w], f32, tag="big", name=f"pbig{i}")
                    for i, w in enumerate(nw)
                ]
                ar = adjt.ap().rearrange("(g j p) m -> g p j m", j=kg, p=128)
                for g in range(kb // kg):
                    at = apool.tile([128, kg, s], f16, tag="adj")
                    (ringA if g % 2 == 0 else ringB).dma_start(at[:], ar[g])
                    for j in range(kg):
                        k = g * kg + j
                        for n, (o, w) in enumerate(zip(no, nw)):
                            nc.tensor.matmul(
                                ps[n][:],
                                hs[:, k, :],
                                at[:, j, o : o + w],
                                start=(k == 0),
                                stop=(k == kb - 1),
                            )
                return ps

            def elu_chunks(ps, dst):
                # dst[:, s] = elu(ps chunks), fp32
                for n, (o, w) in enumerate(zip(no, nw)):
                    neg = tmp.tile([128, 512], f32, tag="neg", name=f"neg{n}")
                    nc.vector.tensor_scalar_min(neg[:, :w], ps[n][:], 0.0)
                    ex = tmp.tile([128, 512], f32, tag="ex", name=f"ex{n}")
                    nc.scalar.activation(ex[:, :w], neg[:, :w], AF.Exp)
                    pm1 = tmp.tile([128, 512], f32, tag="pm1", name=f"pm1{n}")
                    nc.vector.tensor_scalar(
                        pm1[:, :w], ps[n][:], 0.0, -1.0, op0=OP.max, op1=OP.add
                    )
                    nc.vector.tensor_add(dst[:, o : o + w], ex[:, :w], pm1[:, :w])

            # ---- layer 1: h1 replicated (no collective) ----
            # xc group g holds 8 chunk-lhsTs contiguous per partition:
            # xc[g*128+p, ((j*2+a)*128)+m] = xrot.T[a*128+p, (g*8+j)*128+m]
            hs1 = hpool.tile([128, kb, D], f16, tag="hfull")
            xr = xc.ap().rearrange("(g p) q -> g p q", p=128)
            xg = None
            for k in range(kb):
                g, j = divmod(k, 8)
                if j == 0:
                    xg = xcpool.tile([128, 8, 2, 128], f16, tag="xg")
                    (ringA if g % 2 == 0 else ringB).dma_start(
                        xg.rearrange("p j a m -> p (j a m)"), xr[g]
                    )
                ph = pss.tile([128, D], f32, tag="pss", name=f"ph1_{k}")
                nc.tensor.matmul(
                    ph[:], xg[:, j, 0, :], w1s[:, 0, :], start=True, stop=False
                )
                nc.tensor.matmul(
                    ph[:], xg[:, j, 1, :], w1s[:, 1, :], start=False, stop=True
                )
                nc.vector.tensor_copy(hs1[:, k, :], ph[:])
            ps1 = big_layer(hs1)
            x2t = xepool.tile([128, s], f32, tag="xe")
            elu_chunks(ps1, x2t)

            # ---- layer 2 ----
            # own h2 shard (feature-major), start collective, and immediately
            # transpose the local shard into the first rc lhsT chunks
            x2h = xepool.tile([128, s], f16, tag="xeh")
            nc.vector.tensor_copy(x2h[:], x2t[:])
            h2sT = xepool.tile([128, s], f16, tag="h2sT")
            for c in range(rc):
                cs = slice(c * 128, (c + 1) * 128)
                ph2 = pss.tile([128, D], f32, tag="pss", name=f"ph2_{c}")
                # feat-major h2 chunk: W2f.T-stationary @ x2[feat, nodes]
                nc.tensor.matmul(ph2[:], w2s[:], x2h[:, cs], start=True, stop=True)
                nc.vector.tensor_copy(h2sT[:, cs], ph2[:])
            ringC.dma_start(h2b[:], h2sT[:])
            nc.gpsimd.collective_compute(
                "AllGather",
                OP.bypass,
                ins=[h2b.opt()],
                outs=[h2f.opt()],
                replica_groups=rg,
            )
            hs2 = hpool.tile([128, kb, D], f16, tag="hfull")
            for k in range(rc):  # own block: no collective wait
                pt = pss.tile([128, D], f16, tag="pss", name=f"ptl_{k}")
                nc.tensor.transpose(
                    pt[:], h2sT[:, k * 128 : (k + 1) * 128], ident[:]
                )
                nc.vector.tensor_copy(hs2[:, k, :], pt[:])
            # other ranks' blocks: dynamic row offset ((me+1+g) % ncores)*128
            # NOTE: keep these off the sync ring — SP-engine DMAs touching
            # collective-output buffers can hang (test_sync_dma_collective_hang)
            for g in range(ncores - 1):
                with ringB.register(f"hoffr{g}") as hreg:
                    ringB.reg_load(hreg, hoffs[0:1, g : g + 1])
                    off = ringB.snap(hreg, min_val=0, max_val=(ncores - 1) * 128)
                hb = hblkp.tile([128, s], f16, tag="hblk", name=f"hblk{g}")
                ringB.dma_start(hb[:], h2f[bass.ds(off, 128), :])
                for jj in range(rc):
                    k = rc * (1 + g) + jj
                    pt = pss.tile([128, D], f16, tag="pss", name=f"pt_{k}")
                    nc.tensor.transpose(
                        pt[:], hb[:, jj * 128 : (jj + 1) * 128], ident[:]
                    )
                    nc.vector.tensor_copy(hs2[:, k, :], pt[:])
            ps2 = big_layer(hs2)
            x3t = xepool.tile([128, s], f32, tag="xe")
            elu_chunks(ps2, x3t)

            # ---- output layer + log_softmax ----
            outr = out.ap().rearrange("(c p) m -> c p m", p=128)
            zbig = outp.tile([128, rc, C], f32, tag="zbig", bufs=1)
            for c in range(rc):
                py = psy.tile([128, C], f32, tag="psy")
                cs = slice(c * 128, (c + 1) * 128)
                nc.tensor.matmul(py[:], x3t[:, cs], wouts[:], start=True, stop=True)
                nc.vector.tensor_add(zbig[:, c, :], py[:], bbs[:])
            # batched elu over [128, rc*C]
            zf = zbig.rearrange("p c m -> p (c m)")
            negb = tmp.tile([128, rc * C], f32, tag="neg", name="negb")
            nc.vector.tensor_scalar_min(negb[:], zf, 0.0)
            eb = tmp.tile([128, rc * C], f32, tag="ex", name="eb")
            nc.scalar.activation(eb[:], negb[:], AF.Exp)
            pmb = tmp.tile([128, rc * C], f32, tag="pm1", name="pmb")
            nc.vector.tensor_scalar(pmb[:], zf, 0.0, -1.0, op0=OP.max, op1=OP.add)
            zzb = outp.tile([128, rc, C], f32, tag="zzb", bufs=1)
            nc.vector.tensor_add(
                zzb.rearrange("p c m -> p (c m)"), eb[:], pmb[:]
            )
            # batched row-max (negated), then per-chunk exp/lse/final
            negm = stat.tile([128, rc], f32, tag="negm")
            nc.vector.tensor_reduce(
                negm[:], zzb[:], axis=mybir.AxisListType.X, op=OP.max, negate=True
            )
            ssum = stat.tile([128, rc], f32, tag="ssum")
            es = tmp.tile([128, rc * C], f32, tag="neg", name="es")
            esv = es.rearrange("p (c m) -> p c m", m=C)
            for c in range(rc):
                nc.scalar.activation(
                    esv[:, c, :],
                    zzb[:, c, :],
                    AF.Exp,
                    bias=negm[:, c : c + 1],
                    accum_out=ssum[:, c : c + 1],
                )
            lse = stat.tile([128, rc], f32, tag="lse")
            nc.scalar.activation(lse[:], ssum[:], AF.Ln)
            for c in range(rc):
                osb = outp.tile([128, C], f32, tag="osb")
                nc.vector.tensor_scalar(
                    osb[:],
                    zzb[:, c, :],
                    negm[:, c : c + 1],
                    lse[:, c : c + 1],
                    op0=OP.add,
                    op1=OP.subtract,
                )
                ringC.dma_start(outr[c], osb[:])

    nc.compile()
    return nc


def make_in_maps(x, adj, W1, W2, Wout, bout, ncores=NCORES):
    n_total = adj.shape[0]
    s = n_total // ncores
    kb = n_total // 128
    kb8 = kb // 8
    f, d = W1.shape[1], W1.shape[0] * W1.shape[2]
    w1f = np.ascontiguousarray(
        W1.transpose(1, 0, 2).reshape(f, d).astype(np.float16)
    )
    w2f = np.ascontiguousarray(
        W2.transpose(1, 0, 2).reshape(d, d).astype(np.float16)
    )
    woutf = np.ascontiguousarray(Wout.astype(np.float32))
    bbf = np.ascontiguousarray(
        np.broadcast_to(bout.astype(np.float32), (128, Wout.shape[1]))
    )
    adj16 = adj.astype(np.float16)
    x16 = x.astype(np.float16)
    in_maps = []
    for c in range(ncores):
        rows = slice(c * s, (c + 1) * s)
        # rotate contraction rows so this core's own nodes come first
        rot = np.roll(np.arange(n_total), -c * s)
        adjtc = np.ascontiguousarray(adj16[rows][:, rot].T)
        # xc[g*128 + p, ((j*2 + a)*128) + m] = xrot.T[a*128 + p, (g*8 + j)*128 + m]
        xtc = x16[rot].T  # [F, n_total]
        xcf = np.ascontiguousarray(
            xtc.reshape(2, 128, kb8, 8, 128)
            .transpose(2, 1, 3, 0, 4)
            .reshape(kb8 * 128, 8 * f)
        )
        hoffc = np.zeros((1, 8), np.uint32)
        for g in range(ncores - 1):
            hoffc[0, g] = ((c + 1 + g) % ncores) * 128
        in_maps.append(
            {
                "adjt": adjtc,
                "xc": xcf,
                "w1": w1f,
                "w2": w2f,
                "wout": woutf,
                "bb": bbf,
                "hoff": hoffc,
            }
        )
    return in_maps


def kernel(x, adj, W1, W2, Wout, bout):
    from concourse import bass_utils

    x = np.asarray(x)
    adj = np.asarray(adj)
    in_maps = make_in_maps(x, adj, np.asarray(W1), np.asarray(W2),
                           np.asarray(Wout), np.asarray(bout))
    if "nc" not in _nc_cache:
        _nc_cache["nc"] = build_gat_nc()
    res = bass_utils.run_bass_kernel_spmd(
        _nc_cache["nc"], in_maps, core_ids=list(range(NCORES))
    )
    return np.concatenate([r["out"] for r in res.results], axis=0).astype(np.float32)



# revision 10
# speedup vs baseline: 1.6066x; 1.6066x over previous
"""Bass/Trainium2 kernel for the (dead-attention) GAT reference.

Effective math (see reference):
    h1  = x @ W1f                 W1f = W1.transpose(1,0,2).reshape(256,128)
    hp1 = elu(adj @ h1)
    h2  = hp1 @ W2f               W2f = W2.transpose(1,0,2).reshape(128,128)
    hp2 = elu(adj @ h2)
    y   = elu(hp2 @ Wout + bout)
    out = log_softmax(y, axis=1)

Distribution + precision strategy:
  * adj row-sharded 8 ways; each core's 2048x16384 shard is uploaded
    TRANSPOSED, fp8(e4m3), and pre-tiled so every DMA moves 8KB
    contiguous per partition.  The contraction rows are permuted
    [own-rank first, then ranks +1..+7] x [first half, second half] so
    layer 2 can start on locally-available h2 blocks while AllGathers
    fly.
  * Layer 1 streams adj fp8 against an fp16 stationary h1 (replicated
    compute from an interleaved x stream): layer-1 precision dominates
    final error (it is amplified by both all-positive adj matmuls), so
    it stays fp16.  Layer 2 uses fp8 h2 + fp8 adj with DoubleRow
    (2 contraction blocks per PE pass).
  * Each layer is split into two output-column halves.  After half 0 of
    layer 1 finishes, its h2 shard is cast to fp8 and AllGathered while
    half 1 still streams -> collective fully hidden; the second (small)
    AllGather is covered by layer-2 work on already-present blocks.
    The half split also lets the half-0 output stage (elu + Wout +
    log_softmax + store) overlap layer 2's half-1 stream.
"""

import sys

import numpy as np

sys.path.insert(0, "/opt/trn_rl_repo")

N = 16384  # nodes
F = 256  # input features
D = 128  # hidden width (nheads*nhid)
C = 32  # classes
NCORES = 8
S = N // NCORES  # rows per core

_nc_cache = {}


def build_gat_nc(n_total=N, ncores=NCORES, enable_asserts=False, abufs=10):
    """Build the SPMD Bass program (one program, runs on all cores)."""
    from concourse import bacc, bass, masks, mybir, tile

    s = n_total // ncores  # shard rows per core
    sh = s // 2  # output columns per half
    NB = n_total // 128  # contraction blocks
    NBH = NB // 2  # blocks per contraction-half
    BS = NBH // ncores  # blocks per rank segment (per half)
    KG = 4  # contraction pairs per adj tile
    NT = NB // (KG * 2)  # adj tiles per output-half phase
    NXG = NB // 8  # xc groups (8 blocks each)
    RH = sh // 128  # 128-node output chunks per half
    # output chunk widths within a half (PSUM bank = 512 fp32)
    CW = [min(512, sh - o) for o in range(0, sh, 512)]
    CO = list(range(0, sh, 512))
    # first L2 tile that needs remote second-half h2 blocks
    GSTAR = (NBH + BS) // (KG * 2)
    f32 = mybir.dt.float32
    f16 = mybir.dt.float16
    f8 = mybir.dt.float8e4
    AF = mybir.ActivationFunctionType
    OP = mybir.AluOpType
    PM = mybir.MatmulPerfMode

    nc = bacc.Bacc(
        "TRN2",
        target_bir_lowering=False,
        debug=False,
        enable_asserts=enable_asserts,
        num_devices=ncores,
    )

    adjt = nc.dram_tensor("adjt", [2 * NT * 128, KG * 2 * sh], f8, kind="ExternalInput")
    xc = nc.dram_tensor("xc", [NXG * 128, 8 * F], f16, kind="ExternalInput")
    w1 = nc.dram_tensor("w1", [F, D], f16, kind="ExternalInput")
    w2 = nc.dram_tensor("w2", [D, D], f16, kind="ExternalInput")
    wout = nc.dram_tensor("wout", [D, C], f32, kind="ExternalInput")
    bb = nc.dram_tensor("bb", [128, C], f32, kind="ExternalInput")
    # hoff[0, g] = ((rank + 1 + g) % ncores) * 128: gather-block row offsets
    hoff = nc.dram_tensor("hoff", [1, 8], mybir.dt.uint32, kind="ExternalInput")
    out = nc.dram_tensor("out", [s, C], f32, kind="ExternalOutput")

    rg = [list(range(ncores))]

    from contextlib import ExitStack

    with ExitStack() as stack:
        tc = stack.enter_context(tile.TileContext(nc))
        pool = lambda **kw: stack.enter_context(tc.tile_pool(**kw))
        dram = pool(name="dram", bufs=1, space="DRAM")
        const = pool(name="const", bufs=1)
        hs1p = pool(name="hs1p", bufs=1)
        hs2p = pool(name="hs2p", bufs=1)
        apool = pool(name="adjs", bufs=abufs)
        hblkp = pool(name="hblkp", bufs=8)
        xcpool = pool(name="xcp", bufs=2)
        xepool = pool(name="xe", bufs=2)
        x2hp = pool(name="x2hp", bufs=2)
        h2sp = pool(name="h2sp", bufs=2)
        tmp = pool(name="tmp", bufs=2)
        outp = pool(name="outp", bufs=2)
        stat = pool(name="stat", bufs=2)
        psb = pool(name="psb", bufs=2, space="PSUM")
        pss = pool(name="pss", bufs=2, space="PSUM")
        psy = pool(name="psy", bufs=2, space="PSUM")
        if True:
            ringA, ringB, ringC = nc.sync, nc.scalar, nc.gpsimd

            # --- replicated constants (SWDGE so HW rings start streaming) ---
            w1s = const.tile([128, 2, D], f16, tag="w1s")
            ringC.dma_start(w1s[:], w1.ap().rearrange("(a p) m -> p a m", p=128))
            w2s = const.tile([128, D], f16, tag="w2s")
            ringC.dma_start(w2s[:], w2.ap())
            wouts = const.tile([128, C], f32, tag="wouts")
            ringC.dma_start(wouts[:], wout.ap())
            bbs = const.tile([128, C], f32, tag="bbs")
            ringC.dma_start(bbs[:], bb.ap())
            hoffs = const.tile([1, 8], mybir.dt.uint32, tag="hoffs")
            ringC.dma_start(hoffs[:], hoff.ap())
            ident = const.tile([128, 128], f8, tag="ident")
            masks.make_identity(nc, ident[:])

            # --- DRAM bounce buffers for the two half AllGathers (fp8) ---
            h2b = [dram.tile([128, sh], f8, tag="h2b", name=f"h2b{h}") for h in (0, 1)]
            h2f = [
                dram.tile([128 * ncores, sh], f8, tag="h2f", name=f"h2f{h}",
                          addr_space="Shared")
                for h in (0, 1)
            ]

            ar = adjt.ap().rearrange("(t p) m -> t p m", p=128)
            xr = xc.ap().rearrange("(g p) q -> g p q", p=128)
            remote_bufs = {}

            hs1 = hs1p.tile([128, NB, D], f16, tag="hs1")
            hs2 = hs2p.tile([128, NB, D], f8, tag="hs2")

            def stream_tile(layer, hh2, g):
                at = apool.tile([128, KG, 2, sh], f8, tag="adj")
                ring = ringA if g % 2 == 0 else ringB
                ring.dma_start(
                    at.rearrange("p j i m -> p (j i m)"), ar[hh2 * NT + g]
                )
                return at

            def elu_half(ps, dst, hh2):
                # dst[:, hh2 half] = elu(psum chunks), fp32
                for n, (o, w) in enumerate(zip(CO, CW)):
                    neg = tmp.tile([128, 512], f32, tag="neg", name=f"neg{n}")
                    nc.vector.tensor_scalar_min(neg[:, :w], ps[n][:], 0.0)
                    ex = tmp.tile([128, 512], f32, tag="ex", name=f"ex{n}")
                    nc.scalar.activation(ex[:, :w], neg[:, :w], AF.Exp)
                    pm1 = tmp.tile([128, 512], f32, tag="pm1", name=f"pm1{n}")
                    nc.vector.tensor_scalar(
                        pm1[:, :w], ps[n][:], 0.0, -1.0, op0=OP.max, op1=OP.add
                    )
                    nc.vector.tensor_add(
                        dst[:, hh2 * sh + o : hh2 * sh + o + w], ex[:, :w], pm1[:, :w]
                    )

            # ================= layer 1 (+ h1 interleaved, replicated) ======
            x2t = xepool.tile([128, s], f32, tag="xe", name="x2t")
            h2sT = [None, None]
            for hh2 in (0, 1):
                ps = [
                    psb.tile([128, w], f32, tag=f"bg{n}", name=f"ps1_{hh2}_{n}")
                    for n, w in enumerate(CW)
                ]
                for g in range(NT):
                    at = stream_tile(1, hh2, g)
                    if hh2 == 0:
                        xg = xcpool.tile([128, 8, 2, 128], f16, tag="xg")
                        (ringB if g % 2 == 0 else ringA).dma_start(
                            xg.rearrange("p j a m -> p (j a m)"), xr[g]
                        )
                    for j in range(KG):
                        for i in (0, 1):
                            k = g * KG * 2 + j * 2 + i
                            if hh2 == 0:
                                ph = pss.tile([128, D], f32, tag="pss",
                                              name=f"ph1_{k}")
                                nc.tensor.matmul(
                                    ph[:], xg[:, j * 2 + i, 0, :], w1s[:, 0, :],
                                    start=True, stop=False,
                                )
                                nc.tensor.matmul(
                                    ph[:], xg[:, j * 2 + i, 1, :], w1s[:, 1, :],
                                    start=False, stop=True,
                                )
                                nc.vector.tensor_copy(hs1[:, k, :], ph[:])
                            for n, (o, w) in enumerate(zip(CO, CW)):
                                nc.tensor.matmul(
                                    ps[n][:],
                                    hs1[:, k, :],
                                    at[:, j, i, o : o + w],
                                    start=(k == 0),
                                    stop=(k == NB - 1),
                                )
                # ---- boundary hh2: elu, h2 chunk, cast fp8, AllGather ----
                elu_half(ps, x2t, hh2)
                x2h = x2hp.tile([128, sh], f16, tag="x2h", name=f"x2h{hh2}")
                nc.vector.tensor_copy(x2h[:], x2t[:, hh2 * sh : (hh2 + 1) * sh])
                hT = h2sp.tile([128, sh], f8, tag="h2sT", name=f"h2sT{hh2}")
                h2sT[hh2] = hT
                for cb in range(RH):
                    cs = slice(cb * 128, (cb + 1) * 128)
                    ph2 = pss.tile([128, D], f32, tag="pss", name=f"ph2_{hh2}_{cb}")
                    nc.tensor.matmul(ph2[:], w2s[:], x2h[:, cs], start=True, stop=True)
                    nc.vector.tensor_copy(hT[:, cs], ph2[:])
                ringC.dma_start(h2b[hh2][:], hT[:])
                nc.gpsimd.collective_compute(
                    "AllGather",
                    OP.bypass,
                    ins=[h2b[hh2].opt()],
                    outs=[h2f[hh2].opt()],
                    replica_groups=rg,
                )
                # own-rank hs2 blocks for this half: local transposes
                # (fp8 transpose writes 16-bit lanes -> step-2 output view)
                for b in range(BS):
                    kk = hh2 * NBH + b
                    pt = pss.tile([128, 128, 2], f8, tag="pss", name=f"ptl_{kk}")
                    nc.tensor.transpose(
                        pt[:, :, 0], hT[:, b * 128 : (b + 1) * 128], ident[:]
                    )
                    nc.vector.tensor_copy(hs2[:, kk, :], pt[:, :, 0])
                # remote fetches for this half (gpsimd queue, after the AG)
                for g in range(ncores - 1):
                    with ringC.register(f"hoff_{hh2}_{g}") as hreg:
                        ringC.reg_load(hreg, hoffs[0:1, g : g + 1])
                        off = ringC.snap(hreg, min_val=0, max_val=(ncores - 1) * 128)
                    hb = hblkp.tile([128, sh], f8, tag="hblk", name=f"hblk{hh2}_{g}")
                    ringC.dma_start(hb[:], h2f[hh2][bass.ds(off, 128), :])
                    remote_bufs[(hh2, g)] = hb

            # remote transposes, first half (AG0 landed mid layer 1)
            def remote_transposes(hh):
                for g in range(ncores - 1):
                    hb = remote_bufs[(hh, g)]
                    for b in range(BS):
                        kk = hh * NBH + (1 + g) * BS + b
                        pt = pss.tile([128, 128, 2], f8, tag="pss", name=f"pt_{kk}")
                        nc.tensor.transpose(
                            pt[:, :, 0], hb[:, b * 128 : (b + 1) * 128], ident[:]
                        )
                        nc.vector.tensor_copy(hs2[:, kk, :], pt[:, :, 0])

            remote_transposes(0)

            # ================= layer 2 (fp8 DoubleRow) ====================
            x3t = xepool.tile([128, s], f32, tag="xe", name="x3t")

            def out_stage(hh2):
                # output layer + log_softmax for this half of the nodes
                outr = out.ap().rearrange("(c p) m -> c p m", p=128)
                zbig = outp.tile([128, RH, C], f32, tag="zbig", name=f"zbig{hh2}")
                for cb in range(RH):
                    py = psy.tile([128, C], f32, tag="psy", name=f"py{hh2}_{cb}")
                    cs = slice(hh2 * sh + cb * 128, hh2 * sh + (cb + 1) * 128)
                    nc.tensor.matmul(py[:], x3t[:, cs], wouts[:], start=True, stop=True)
                    nc.vector.tensor_add(zbig[:, cb, :], py[:], bbs[:])
                zf = zbig.rearrange("p c m -> p (c m)")
                negb = tmp.tile([128, RH * C], f32, tag="negB", name=f"negb{hh2}")
                nc.vector.tensor_scalar_min(negb[:], zf, 0.0)
                eb = tmp.tile([128, RH * C], f32, tag="exB", name=f"eb{hh2}")
                nc.scalar.activation(eb[:], negb[:], AF.Exp)
                pmb = tmp.tile([128, RH * C], f32, tag="pmB", name=f"pmb{hh2}")
                nc.vector.tensor_scalar(pmb[:], zf, 0.0, -1.0, op0=OP.max, op1=OP.add)
                zzb = outp.tile([128, RH, C], f32, tag="zzb", name=f"zzb{hh2}")
                nc.vector.tensor_add(zzb.rearrange("p c m -> p (c m)"), eb[:], pmb[:])
                negm = stat.tile([128, RH], f32, tag="negm", name=f"negm{hh2}")
                nc.vector.tensor_reduce(
                    negm[:], zzb[:], axis=mybir.AxisListType.X, op=OP.max, negate=True
                )
                ssum = stat.tile([128, RH], f32, tag="ssum", name=f"ssum{hh2}")
                es = tmp.tile([128, RH * C], f32, tag="negB", name=f"es{hh2}")
                esv = es.rearrange("p (c m) -> p c m", m=C)
                for cb in range(RH):
                    nc.scalar.activation(
                        esv[:, cb, :],
                        zzb[:, cb, :],
                        AF.Exp,
                        bias=negm[:, cb : cb + 1],
                        accum_out=ssum[:, cb : cb + 1],
                    )
                lse = stat.tile([128, RH], f32, tag="lse", name=f"lse{hh2}")
                nc.scalar.activation(lse[:], ssum[:], AF.Ln)
                for cb in range(RH):
                    osb = outp.tile([128, C], f32, tag="osb", name=f"osb{hh2}_{cb}")
                    nc.vector.tensor_scalar(
                        osb[:],
                        zzb[:, cb, :],
                        negm[:, cb : cb + 1],
                        lse[:, cb : cb + 1],
                        op0=OP.add,
                        op1=OP.subtract,
                    )
                    ringC.dma_start(outr[hh2 * RH + cb], osb[:])

            for hh2 in (0, 1):
                ps = [
                    psb.tile([128, w], f32, tag=f"bg{n}", name=f"ps2_{hh2}_{n}")
                    for n, w in enumerate(CW)
                ]
                for g in range(NT):
                    if hh2 == 0 and g == GSTAR:
                        # second-half remote blocks: transposes placed here so
                        # the PE has covered the AG1 latency with earlier pairs
                        remote_transposes(1)
                    at = stream_tile(2, hh2, g)
                    for j in range(KG):
                        kp = g * KG + j
                        for n, (o, w) in enumerate(zip(CO, CW)):
                            nc.tensor.matmul(
                                ps[n][:],
                                hs2[:, 2 * kp : 2 * kp + 2, :],
                                at[:, j, :, o : o + w],
                                start=(kp == 0),
                                stop=(kp == NB // 2 - 1),
                                perf_mode=PM.DoubleRow,
                            )
                elu_half(ps, x3t, hh2)
                out_stage(hh2)

    nc.compile()
    return nc


def make_in_maps(x, adj, W1, W2, Wout, bout, ncores=NCORES):
    import ml_dtypes

    E8 = ml_dtypes.float8_e4m3  # TRN fp8e4 (IEEE-ish, max +-240)
    n_total = adj.shape[0]
    s = n_total // ncores
    sh = s // 2
    NB = n_total // 128
    KG = 4
    NT = NB // (KG * 2)
    NXG = NB // 8
    f, d = W1.shape[1], W1.shape[0] * W1.shape[2]
    w1f = np.ascontiguousarray(W1.transpose(1, 0, 2).reshape(f, d).astype(np.float16))
    w2f = np.ascontiguousarray(W2.transpose(1, 0, 2).reshape(d, d).astype(np.float16))
    woutf = np.ascontiguousarray(Wout.astype(np.float32))
    bbf = np.ascontiguousarray(
        np.broadcast_to(bout.astype(np.float32), (128, Wout.shape[1]))
    )
    adj8 = adj.astype(E8)
    x16 = x.astype(np.float16)
    in_maps = []
    for c in range(ncores):
        rows = slice(c * s, (c + 1) * s)
        # contraction permutation: [half 0 | half 1] x [rank c, c+1, ..]
        perm = np.concatenate(
            [
                np.arange(((c + rr) % ncores) * s + hh * sh,
                          ((c + rr) % ncores) * s + (hh + 1) * sh)
                for hh in (0, 1)
                for rr in range(ncores)
            ]
        )
        adjtc = adj8[rows][:, perm].T  # [n_total (perm), s]
        # pre-tiled fp8 layout: per output half, [NT,128,KG,2,sh] -> rows
        halves = []
        for hh2 in (0, 1):
            Ah = adjtc[:, hh2 * sh : (hh2 + 1) * sh]
            halves.append(
                Ah.reshape(NT, KG, 2, 128, sh)
                .transpose(0, 3, 1, 2, 4)
                .reshape(NT * 128, KG * 2 * sh)
            )
        adjt_np = np.ascontiguousarray(np.concatenate(halves, axis=0))
        # xc[g*128 + p, ((j*2 + a)*128) + m] = xperm.T[a*128 + p, (g*8 + j)*128 + m]
        xtc = x16[perm].T  # [F, n_total]
        xcf = np.ascontiguousarray(
            xtc.reshape(2, 128, NXG, 8, 128)
            .transpose(2, 1, 3, 0, 4)
            .reshape(NXG * 128, 8 * f)
        )
        hoffc = np.zeros((1, 8), np.uint32)
        for g in range(ncores - 1):
            hoffc[0, g] = ((c + 1 + g) % ncores) * 128
        in_maps.append(
            {
                "adjt": adjt_np,
                "xc": xcf,
                "w1": w1f,
                "w2": w2f,
                "wout": woutf,
                "bb": bbf,
                "hoff": hoffc,
            }
        )
    return in_maps


def kernel(x, adj, W1, W2, Wout, bout):
    from concourse import bass_utils

    x = np.asarray(x)
    adj = np.asarray(adj)
    in_maps = make_in_maps(x, adj, np.asarray(W1), np.asarray(W2),
                           np.asarray(Wout), np.asarray(bout))
    if "nc" not in _nc_cache:
        _nc_cache["nc"] = build_gat_nc()
    res = bass_utils.run_bass_kernel_spmd(
        _nc_cache["nc"], in_maps, core_ids=list(range(NCORES))
    )
    return np.concatenate([r["out"] for r in res.results], axis=0).astype(np.float32)


# revision 13
# speedup vs baseline: 1.6721x; 1.0408x over previous
"""Bass/Trainium2 kernel for the (dead-attention) GAT reference.

Effective math (see reference):
    h1  = x @ W1f                 W1f = W1.transpose(1,0,2).reshape(256,128)
    hp1 = elu(adj @ h1)
    h2  = hp1 @ W2f               W2f = W2.transpose(1,0,2).reshape(128,128)
    hp2 = elu(adj @ h2)
    y   = elu(hp2 @ Wout + bout)
    out = log_softmax(y, axis=1)

Distribution + precision strategy:
  * adj row-sharded 8 ways; each core's 2048x16384 shard is uploaded
    TRANSPOSED, fp8(e4m3), and pre-tiled so every DMA moves 8KB
    contiguous per partition.  The contraction rows are permuted
    [own-rank first, then ranks +1..+7] x [first half, second half] so
    layer 2 can start on locally-available h2 blocks while AllGathers
    fly.
  * Layer 1 streams adj fp8 against an fp16 stationary h1 (replicated
    compute from an interleaved x stream): layer-1 precision dominates
    final error (it is amplified by both all-positive adj matmuls), so
    it stays fp16.  Layer 2 uses fp8 h2 + fp8 adj with DoubleRow
    (2 contraction blocks per PE pass).
  * Each layer is split into two output-column halves.  After half 0 of
    layer 1 finishes, its h2 shard is cast to fp8 and AllGathered while
    half 1 still streams -> collective fully hidden; the second (small)
    AllGather is covered by layer-2 work on already-present blocks.
    The half split also lets the half-0 output stage (elu + Wout +
    log_softmax + store) overlap layer 2's half-1 stream.
"""

import sys

import numpy as np

sys.path.insert(0, "/opt/trn_rl_repo")

N = 16384  # nodes
F = 256  # input features
D = 128  # hidden width (nheads*nhid)
C = 32  # classes
NCORES = 8
S = N // NCORES  # rows per core

_nc_cache = {}


def build_gat_nc(n_total=N, ncores=NCORES, enable_asserts=False, abufs=12):
    """Build the SPMD Bass program (one program, runs on all cores)."""
    from concourse import bacc, bass, masks, mybir, tile

    s = n_total // ncores  # shard rows per core
    sh = s // 2  # output columns per half
    NB = n_total // 128  # contraction blocks
    NBH = NB // 2  # blocks per contraction-half
    BS = NBH // ncores  # blocks per rank segment (per half)
    KG = 4  # contraction pairs per adj tile
    NT = NB // (KG * 2)  # adj tiles per output-half phase
    NXG = NB // 8  # xc groups (8 blocks each)
    RH = sh // 128  # 128-node output chunks per half
    # output chunk widths within a half (PSUM bank = 512 fp32)
    CW = [min(512, sh - o) for o in range(0, sh, 512)]
    CO = list(range(0, sh, 512))
    # first L2 tile that needs remote second-half h2 blocks
    GSTAR = (NBH + BS) // (KG * 2)
    f32 = mybir.dt.float32
    f16 = mybir.dt.float16
    f8 = mybir.dt.float8e4
    AF = mybir.ActivationFunctionType
    OP = mybir.AluOpType
    PM = mybir.MatmulPerfMode

    nc = bacc.Bacc(
        "TRN2",
        target_bir_lowering=False,
        debug=False,
        enable_asserts=enable_asserts,
        num_devices=ncores,
    )

    adjt = nc.dram_tensor("adjt", [2 * NT * 128, KG * 2 * sh], f8, kind="ExternalInput")
    xc = nc.dram_tensor("xc", [NXG * 128, 8 * F], f16, kind="ExternalInput")
    w1 = nc.dram_tensor("w1", [F, D], f16, kind="ExternalInput")
    w2 = nc.dram_tensor("w2", [D, D], f16, kind="ExternalInput")
    wout = nc.dram_tensor("wout", [D, C], f32, kind="ExternalInput")
    bb = nc.dram_tensor("bb", [128, C], f32, kind="ExternalInput")
    # hoff[0, g] = ((rank + 1 + g) % ncores) * 128: gather-block row offsets
    hoff = nc.dram_tensor("hoff", [1, 8], mybir.dt.uint32, kind="ExternalInput")
    out = nc.dram_tensor("out", [s, C], f32, kind="ExternalOutput")

    rg = [list(range(ncores))]

    from contextlib import ExitStack

    with ExitStack() as stack:
        tc = stack.enter_context(tile.TileContext(nc))
        pool = lambda **kw: stack.enter_context(tc.tile_pool(**kw))
        dram = pool(name="dram", bufs=1, space="DRAM")
        const = pool(name="const", bufs=1)
        hs1p = pool(name="hs1p", bufs=1)
        hs2p = pool(name="hs2p", bufs=1)
        apool = pool(name="adjs", bufs=abufs)
        hblkp = pool(name="hblkp", bufs=8)
        xcpool = pool(name="xcp", bufs=2)
        xepool = pool(name="xe", bufs=2)
        x2hp = pool(name="x2hp", bufs=2)
        h2sp = pool(name="h2sp", bufs=2)
        tmp = pool(name="tmp", bufs=2)
        outp = pool(name="outp", bufs=2)
        stat = pool(name="stat", bufs=2)
        psb = pool(name="psb", bufs=2, space="PSUM")
        pss = pool(name="pss", bufs=2, space="PSUM")
        psy = pool(name="psy", bufs=2, space="PSUM")
        if True:
            ringA, ringB, ringC = nc.sync, nc.scalar, nc.gpsimd

            # --- replicated constants (SWDGE so HW rings start streaming) ---
            w1s = const.tile([128, 2, D], f16, tag="w1s")
            ringC.dma_start(w1s[:], w1.ap().rearrange("(a p) m -> p a m", p=128))
            w2s = const.tile([128, D], f16, tag="w2s")
            ringC.dma_start(w2s[:], w2.ap())
            wouts = const.tile([128, C], f32, tag="wouts")
            ringC.dma_start(wouts[:], wout.ap())
            bbs = const.tile([128, C], f32, tag="bbs")
            ringC.dma_start(bbs[:], bb.ap())
            hoffs = const.tile([1, 8], mybir.dt.uint32, tag="hoffs")
            ringC.dma_start(hoffs[:], hoff.ap())
            ident = const.tile([128, 128], f8, tag="ident")
            masks.make_identity(nc, ident[:])

            # --- DRAM bounce buffers for the two half AllGathers (fp8) ---
            h2b = [dram.tile([128, sh], f8, tag="h2b", name=f"h2b{h}") for h in (0, 1)]
            h2f = [
                dram.tile([128 * ncores, sh], f8, tag="h2f", name=f"h2f{h}",
                          addr_space="Shared")
                for h in (0, 1)
            ]

            ar = adjt.ap().rearrange("(t p) m -> t p m", p=128)
            xr = xc.ap().rearrange("(g p) q -> g p q", p=128)
            remote_bufs = {}

            hs1 = hs1p.tile([128, NB, D], f16, tag="hs1")
            hs2 = hs2p.tile([128, NB, D], f8, tag="hs2")

            def stream_tile(layer, hh2, g):
                at = apool.tile([128, KG, 2, sh], f8, tag="adj")
                ring = ringA if g % 2 == 0 else ringB
                ring.dma_start(
                    at.rearrange("p j i m -> p (j i m)"), ar[hh2 * NT + g]
                )
                return at

            def elu_half(ps, dst, hh2):
                # dst[:, hh2 half] = elu(psum chunks), fp32
                for n, (o, w) in enumerate(zip(CO, CW)):
                    neg = tmp.tile([128, 512], f32, tag="neg", name=f"neg{n}")
                    nc.vector.tensor_scalar_min(neg[:, :w], ps[n][:], 0.0)
                    ex = tmp.tile([128, 512], f32, tag="ex", name=f"ex{n}")
                    nc.scalar.activation(ex[:, :w], neg[:, :w], AF.Exp)
                    pm1 = tmp.tile([128, 512], f32, tag="pm1", name=f"pm1{n}")
                    nc.vector.tensor_scalar(
                        pm1[:, :w], ps[n][:], 0.0, -1.0, op0=OP.max, op1=OP.add
                    )
                    nc.vector.tensor_add(
                        dst[:, hh2 * sh + o : hh2 * sh + o + w], ex[:, :w], pm1[:, :w]
                    )

            # ================= layer 1 (+ h1 interleaved, replicated) ======
            x2t = xepool.tile([128, s], f32, tag="xe", name="x2t")
            h2sT = [None, None]
            for hh2 in (0, 1):
                ps = [
                    psb.tile([128, w], f32, tag=f"bg{n}", name=f"ps1_{hh2}_{n}")
                    for n, w in enumerate(CW)
                ]
                for g in range(NT):
                    at = stream_tile(1, hh2, g)
                    if hh2 == 0:
                        xg = xcpool.tile([128, 8, 2, 128], f16, tag="xg")
                        (ringB if g % 2 == 0 else ringA).dma_start(
                            xg.rearrange("p j a m -> p (j a m)"), xr[g]
                        )
                    for j in range(KG):
                        for i in (0, 1):
                            k = g * KG * 2 + j * 2 + i
                            if hh2 == 0:
                                ph = pss.tile([128, D], f32, tag="pss",
                                              name=f"ph1_{k}")
                                nc.tensor.matmul(
                                    ph[:], xg[:, j * 2 + i, 0, :], w1s[:, 0, :],
                                    start=True, stop=False,
                                )
                                nc.tensor.matmul(
                                    ph[:], xg[:, j * 2 + i, 1, :], w1s[:, 1, :],
                                    start=False, stop=True,
                                )
                                nc.vector.tensor_copy(hs1[:, k, :], ph[:])
                            for n, (o, w) in enumerate(zip(CO, CW)):
                                nc.tensor.matmul(
                                    ps[n][:],
                                    hs1[:, k, :],
                                    at[:, j, i, o : o + w],
                                    start=(k == 0),
                                    stop=(k == NB - 1),
                                )
                # ---- boundary hh2: elu, h2 chunk, cast fp8, AllGather ----
                elu_half(ps, x2t, hh2)
                x2h = x2hp.tile([128, sh], f16, tag="x2h", name=f"x2h{hh2}")
                nc.vector.tensor_copy(x2h[:], x2t[:, hh2 * sh : (hh2 + 1) * sh])
                hT = h2sp.tile([128, sh], f8, tag="h2sT", name=f"h2sT{hh2}")
                h2sT[hh2] = hT
                for cb in range(RH):
                    cs = slice(cb * 128, (cb + 1) * 128)
                    ph2 = pss.tile([128, D], f32, tag="pss", name=f"ph2_{hh2}_{cb}")
                    nc.tensor.matmul(ph2[:], w2s[:], x2h[:, cs], start=True, stop=True)
                    nc.vector.tensor_copy(hT[:, cs], ph2[:])
                ringC.dma_start(h2b[hh2][:], hT[:])
                nc.gpsimd.collective_compute(
                    "AllGather",
                    OP.bypass,
                    ins=[h2b[hh2].opt()],
                    outs=[h2f[hh2].opt()],
                    replica_groups=rg,
                )
                # own-rank hs2 blocks for this half: local transposes
                # (fp8 transpose writes 16-bit lanes -> step-2 output view)
                for b in range(BS):
                    kk = hh2 * NBH + b
                    pt = pss.tile([128, 128, 2], f8, tag="pss", name=f"ptl_{kk}")
                    nc.tensor.transpose(
                        pt[:, :, 0], hT[:, b * 128 : (b + 1) * 128], ident[:]
                    )
                    nc.vector.tensor_copy(hs2[:, kk, :], pt[:, :, 0])
                # remote fetches for this half (gpsimd queue, after the AG)
                for g in range(ncores - 1):
                    with ringC.register(f"hoff_{hh2}_{g}") as hreg:
                        ringC.reg_load(hreg, hoffs[0:1, g : g + 1])
                        off = ringC.snap(hreg, min_val=0, max_val=(ncores - 1) * 128)
                    hb = hblkp.tile([128, sh], f8, tag="hblk", name=f"hblk{hh2}_{g}")
                    ringC.dma_start(hb[:], h2f[hh2][bass.ds(off, 128), :])
                    remote_bufs[(hh2, g)] = hb

            # remote transposes, first half (AG0 landed mid layer 1)
            def remote_transposes(hh):
                for g in range(ncores - 1):
                    hb = remote_bufs[(hh, g)]
                    for b in range(BS):
                        kk = hh * NBH + (1 + g) * BS + b
                        pt = pss.tile([128, 128, 2], f8, tag="pss", name=f"pt_{kk}")
                        nc.tensor.transpose(
                            pt[:, :, 0], hb[:, b * 128 : (b + 1) * 128], ident[:]
                        )
                        nc.vector.tensor_copy(hs2[:, kk, :], pt[:, :, 0])

            remote_transposes(0)

            # ================= layer 2 (fp8 DoubleRow) ====================
            x3t = xepool.tile([128, s], f32, tag="xe", name="x3t")

            def out_stage(hh2):
                # output layer + log_softmax for this half of the nodes
                outr = out.ap().rearrange("(c p) m -> c p m", p=128)
                zbig = outp.tile([128, RH, C], f32, tag="zbig", name=f"zbig{hh2}")
                for cb in range(RH):
                    py = psy.tile([128, C], f32, tag="psy", name=f"py{hh2}_{cb}")
                    cs = slice(hh2 * sh + cb * 128, hh2 * sh + (cb + 1) * 128)
                    nc.tensor.matmul(py[:], x3t[:, cs], wouts[:], start=True, stop=True)
                    nc.vector.tensor_add(zbig[:, cb, :], py[:], bbs[:])
                zf = zbig.rearrange("p c m -> p (c m)")
                negb = tmp.tile([128, RH * C], f32, tag="negB", name=f"negb{hh2}")
                nc.vector.tensor_scalar_min(negb[:], zf, 0.0)
                eb = tmp.tile([128, RH * C], f32, tag="exB", name=f"eb{hh2}")
                nc.scalar.activation(eb[:], negb[:], AF.Exp)
                pmb = tmp.tile([128, RH * C], f32, tag="pmB", name=f"pmb{hh2}")
                nc.vector.tensor_scalar(pmb[:], zf, 0.0, -1.0, op0=OP.max, op1=OP.add)
                zzb = outp.tile([128, RH, C], f32, tag="zzb", name=f"zzb{hh2}")
                nc.vector.tensor_add(zzb.rearrange("p c m -> p (c m)"), eb[:], pmb[:])
                negm = stat.tile([128, RH], f32, tag="negm", name=f"negm{hh2}")
                nc.vector.tensor_reduce(
                    negm[:], zzb[:], axis=mybir.AxisListType.X, op=OP.max, negate=True
                )
                ssum = stat.tile([128, RH], f32, tag="ssum", name=f"ssum{hh2}")
                es = tmp.tile([128, RH * C], f32, tag="negB", name=f"es{hh2}")
                esv = es.rearrange("p (c m) -> p c m", m=C)
                for cb in range(RH):
                    nc.scalar.activation(
                        esv[:, cb, :],
                        zzb[:, cb, :],
                        AF.Exp,
                        bias=negm[:, cb : cb + 1],
                        accum_out=ssum[:, cb : cb + 1],
                    )
                lse = stat.tile([128, RH], f32, tag="lse", name=f"lse{hh2}")
                nc.scalar.activation(lse[:], ssum[:], AF.Ln)
                for cb in range(RH):
                    osb = outp.tile([128, C], f32, tag="osb", name=f"osb{hh2}_{cb}")
                    nc.vector.tensor_scalar(
                        osb[:],
                        zzb[:, cb, :],
                        negm[:, cb : cb + 1],
                        lse[:, cb : cb + 1],
                        op0=OP.add,
                        op1=OP.subtract,
                    )
                    # stores on the HWDGE rings (SWDGE serializes ~1.7us each)
                    (ringA if cb % 2 == 0 else ringB).dma_start(
                        outr[hh2 * RH + cb], osb[:]
                    )

            # both output-half phases accumulate concurrently; all tiles whose
            # pairs are locally available (first contraction half + own blocks)
            # stream first, maximizing the runway that hides the second
            # AllGather before any remote-second-half block is touched.
            ps2 = {
                hh2: [
                    psb.tile([128, w], f32, tag=f"bg{n}", name=f"ps2_{hh2}_{n}")
                    for n, w in enumerate(CW)
                ]
                for hh2 in (0, 1)
            }
            segs = [(0, 0, GSTAR), (1, 0, GSTAR), (0, GSTAR, NT), (1, GSTAR, NT)]
            for si, (hh2, g0, g1) in enumerate(segs):
                if si == 2:
                    # second-half remote blocks: transposes placed here so the
                    # PE has covered the AG1 latency with earlier pairs
                    remote_transposes(1)
                for g in range(g0, g1):
                    at = stream_tile(2, hh2, g)
                    for j in range(KG):
                        kp = g * KG + j
                        for n, (o, w) in enumerate(zip(CO, CW)):
                            nc.tensor.matmul(
                                ps2[hh2][n][:],
                                hs2[:, 2 * kp : 2 * kp + 2, :],
                                at[:, j, :, o : o + w],
                                start=(kp == 0),
                                stop=(kp == NB // 2 - 1),
                                perf_mode=PM.DoubleRow,
                            )
                if si == 2:
                    elu_half(ps2[0], x3t, 0)
                    out_stage(0)
                elif si == 3:
                    elu_half(ps2[1], x3t, 1)
                    out_stage(1)

    nc.compile()
    return nc


def make_in_maps(x, adj, W1, W2, Wout, bout, ncores=NCORES):
    import ml_dtypes

    E8 = ml_dtypes.float8_e4m3  # TRN fp8e4 (IEEE-ish, max +-240)
    n_total = adj.shape[0]
    s = n_total // ncores
    sh = s // 2
    NB = n_total // 128
    KG = 4
    NT = NB // (KG * 2)
    NXG = NB // 8
    f, d = W1.shape[1], W1.shape[0] * W1.shape[2]
    w1f = np.ascontiguousarray(W1.transpose(1, 0, 2).reshape(f, d).astype(np.float16))
    w2f = np.ascontiguousarray(W2.transpose(1, 0, 2).reshape(d, d).astype(np.float16))
    woutf = np.ascontiguousarray(Wout.astype(np.float32))
    bbf = np.ascontiguousarray(
        np.broadcast_to(bout.astype(np.float32), (128, Wout.shape[1]))
    )
    adj8 = adj.astype(E8)
    x16 = x.astype(np.float16)
    in_maps = []
    for c in range(ncores):
        rows = slice(c * s, (c + 1) * s)
        # contraction permutation: [half 0 | half 1] x [rank c, c+1, ..]
        perm = np.concatenate(
            [
                np.arange(((c + rr) % ncores) * s + hh * sh,
                          ((c + rr) % ncores) * s + (hh + 1) * sh)
                for hh in (0, 1)
                for rr in range(ncores)
            ]
        )
        adjtc = adj8[rows][:, perm].T  # [n_total (perm), s]
        # pre-tiled fp8 layout: per output half, [NT,128,KG,2,sh] -> rows
        halves = []
        for hh2 in (0, 1):
            Ah = adjtc[:, hh2 * sh : (hh2 + 1) * sh]
            halves.append(
                Ah.reshape(NT, KG, 2, 128, sh)
                .transpose(0, 3, 1, 2, 4)
                .reshape(NT * 128, KG * 2 * sh)
            )
        adjt_np = np.ascontiguousarray(np.concatenate(halves, axis=0))
        # xc[g*128 + p, ((j*2 + a)*128) + m] = xperm.T[a*128 + p, (g*8 + j)*128 + m]
        xtc = x16[perm].T  # [F, n_total]
        xcf = np.ascontiguousarray(
            xtc.reshape(2, 128, NXG, 8, 128)
            .transpose(2, 1, 3, 0, 4)
            .reshape(NXG * 128, 8 * f)
        )
        hoffc = np.zeros((1, 8), np.uint32)
        for g in range(ncores - 1):
            hoffc[0, g] = ((c + 1 + g) % ncores) * 128
        in_maps.append(
            {
                "adjt": adjt_np,
                "xc": xcf,
                "w1": w1f,
                "w2": w2f,
                "wout": woutf,
                "bb": bbf,
                "hoff": hoffc,
            }
        )
    return in_maps


def kernel(x, adj, W1, W2, Wout, bout):
    from concourse import bass_utils

    x = np.asarray(x)
    adj = np.asarray(adj)
    in_maps = make_in_maps(x, adj, np.asarray(W1), np.asarray(W2),
                           np.asarray(Wout), np.asarray(bout))
    if "nc" not in _nc_cache:
        _nc_cache["nc"] = build_gat_nc()
    res = bass_utils.run_bass_kernel_spmd(
        _nc_cache["nc"], in_maps, core_ids=list(range(NCORES))
    )
    return np.concatenate([r["out"] for r in res.results], axis=0).astype(np.float32)


# revision 14
# speedup vs baseline: 1.7387x; 1.0398x over previous
"""Bass/Trainium2 kernel for the (dead-attention) GAT reference.

Effective math (see reference):
    h1  = x @ W1f                 W1f = W1.transpose(1,0,2).reshape(256,128)
    hp1 = elu(adj @ h1)
    h2  = hp1 @ W2f               W2f = W2.transpose(1,0,2).reshape(128,128)
    hp2 = elu(adj @ h2)
    y   = elu(hp2 @ Wout + bout)
    out = log_softmax(y, axis=1)

Distribution + precision strategy:
  * adj row-sharded 8 ways; each core's 2048x16384 shard is uploaded
    TRANSPOSED, fp8(e4m3), pre-tiled so every DMA moves 8KB contiguous
    per partition.  Contraction rows are permuted [own rank first, then
    +1..+7] x [first node-half, second node-half] so layer 2 starts on
    locally-available h2 blocks while AllGathers fly.
  * h1 is computed replicated from an fp8 x stream with one DoubleRow
    matmul per 128-node block (W1 pre-scaled by 16 to dodge fp8
    subnormals; the PSUM->SBUF cast divides it back out).  The h1
    STATIONARY stays fp16: layer-1 h precision dominates final error
    (amplified by both all-positive adj matmuls).  h1 for tile g+1 is
    emitted ahead of tile g's adj matmuls so the PE never waits on the
    PSUM->SBUF cast.
  * Layer 2 uses fp8 h2 x fp8 adj with DoubleRow (2 blocks/pass).  h2
    is exchanged NODE-major: own blocks are PE-transposed before the
    fp8 AllGather, and remote blocks are used directly as matmul
    stationaries from the gathered buffer - no unpack transposes.
  * Each layer is split into two output-column halves; the half-0 h2
    AllGather flies under layer 1's half-1 stream.  Layer 2 streams all
    locally-satisfiable tiles of BOTH halves first (~40us runway) to
    cover the second AllGather, and half 0's output stage (elu + Wout +
    log_softmax + store) overlaps half 1's stream.
"""

import sys

import numpy as np

sys.path.insert(0, "/opt/trn_rl_repo")

N = 16384  # nodes
F = 256  # input features
D = 128  # hidden width (nheads*nhid)
C = 32  # classes
NCORES = 8
S = N // NCORES  # rows per core

_nc_cache = {}


def build_gat_nc(n_total=N, ncores=NCORES, enable_asserts=False, abufs=12):
    """Build the SPMD Bass program (one program, runs on all cores)."""
    from contextlib import ExitStack

    from concourse import bacc, bass, masks, mybir, tile

    s = n_total // ncores  # shard rows per core
    sh = s // 2  # output columns per half
    NB = n_total // 128  # contraction blocks
    NBH = NB // 2  # blocks per contraction-half
    BS = NBH // ncores  # blocks per rank segment (per half)
    assert BS % 2 == 0, "pairs must not straddle rank segments"
    KG = 4  # contraction pairs per adj tile
    NT = NB // (KG * 2)  # adj tiles per output-half phase
    NXG = NB // 8  # xc groups (8 blocks each)
    RH = sh // 128  # 128-node output chunks per half
    CW = [min(512, sh - o) for o in range(0, sh, 512)]
    CO = list(range(0, sh, 512))
    # first L2 tile that needs remote second-half h2 blocks
    GSTAR = (NBH + BS) // (KG * 2)
    f32 = mybir.dt.float32
    f16 = mybir.dt.float16
    f8 = mybir.dt.float8e4
    AF = mybir.ActivationFunctionType
    OP = mybir.AluOpType
    PM = mybir.MatmulPerfMode

    nc = bacc.Bacc(
        "TRN2",
        target_bir_lowering=False,
        debug=False,
        enable_asserts=enable_asserts,
        num_devices=ncores,
    )

    adjt = nc.dram_tensor("adjt", [2 * NT * 128, KG * 2 * sh], f8, kind="ExternalInput")
    xc = nc.dram_tensor("xc", [NXG * 128, 8 * F], f8, kind="ExternalInput")
    w1 = nc.dram_tensor("w1", [F, D], f8, kind="ExternalInput")  # pre-scaled x16
    w2 = nc.dram_tensor("w2", [D, D], f16, kind="ExternalInput")
    wout = nc.dram_tensor("wout", [D, C], f32, kind="ExternalInput")
    bb = nc.dram_tensor("bb", [128, C], f32, kind="ExternalInput")
    # hoff[0, g] = ((rank + 1 + g) % ncores) * 128: gather-block row offsets
    hoff = nc.dram_tensor("hoff", [1, 8], mybir.dt.uint32, kind="ExternalInput")
    out = nc.dram_tensor("out", [s, C], f32, kind="ExternalOutput")

    rg = [list(range(ncores))]

    with ExitStack() as stack:
        tc = stack.enter_context(tile.TileContext(nc))
        pool = lambda **kw: stack.enter_context(tc.tile_pool(**kw))
        dram = pool(name="dram", bufs=1, space="DRAM")
        const = pool(name="const", bufs=1)
        hs1p = pool(name="hs1p", bufs=1)
        apool = pool(name="adjs", bufs=abufs)
        hblkp = pool(name="hblkp", bufs=2 * (ncores - 1))
        xcpool = pool(name="xcp", bufs=3)
        xepool = pool(name="xe", bufs=2)
        x2hp = pool(name="x2hp", bufs=2)
        h2sp = pool(name="h2sp", bufs=2)
        h2np = pool(name="h2np", bufs=2)
        tmp = pool(name="tmp", bufs=2)
        outp = pool(name="outp", bufs=4)
        stat = pool(name="stat", bufs=2)
        psb = pool(name="psb", bufs=2, space="PSUM")
        pss = pool(name="pss", bufs=3, space="PSUM")
        psy = pool(name="psy", bufs=1, space="PSUM")
        if True:
            ringA, ringB, ringC = nc.sync, nc.scalar, nc.gpsimd

            # --- replicated constants (SWDGE so HW rings start streaming) ---
            w1s = const.tile([128, 2, D], f8, tag="w1s")
            ringC.dma_start(w1s[:], w1.ap().rearrange("(a p) m -> p a m", p=128))
            w2s = const.tile([128, D], f16, tag="w2s")
            ringC.dma_start(w2s[:], w2.ap())
            wouts = const.tile([128, C], f32, tag="wouts")
            ringC.dma_start(wouts[:], wout.ap())
            bbs = const.tile([128, C], f32, tag="bbs")
            ringC.dma_start(bbs[:], bb.ap())
            hoffs = const.tile([1, 8], mybir.dt.uint32, tag="hoffs")
            ringC.dma_start(hoffs[:], hoff.ap())
            ident = const.tile([128, 128], f8, tag="ident")
            masks.make_identity(nc, ident[:])

            # --- DRAM bounce buffers for the two half AllGathers (fp8) ---
            h2b = [dram.tile([128, BS * D], f8, tag="h2b", name=f"h2b{h}")
                   for h in (0, 1)]
            h2f = [
                dram.tile([128 * ncores, BS * D], f8, tag="h2f", name=f"h2f{h}",
                          addr_space="Shared")
                for h in (0, 1)
            ]

            ar = adjt.ap().rearrange("(t p) m -> t p m", p=128)
            xr = xc.ap().rearrange("(g p) q -> g p q", p=128)
            remote_bufs = {}
            h2n_t = [None, None]

            hs1 = hs1p.tile([128, NB, D], f16, tag="hs1")

            def stream_tile(hh2, g):
                at = apool.tile([128, KG, 2, sh], f8, tag="adj")
                ring = ringA if g % 2 == 0 else ringB
                ring.dma_start(
                    at.rearrange("p j i m -> p (j i m)"), ar[hh2 * NT + g]
                )
                return at

            xg_t = {}

            def load_xg(g):
                if 0 <= g < NXG:
                    t = xcpool.tile([128, 8, 2, 128], f8, tag="xg", name=f"xg{g}")
                    (ringB if g % 2 == 0 else ringA).dma_start(
                        t.rearrange("p j a m -> p (j a m)"), xr[g]
                    )
                    xg_t[g] = t

            def h1_block(k):
                # one DoubleRow matmul: both 128-feature halves in one pass
                g, b = divmod(k, 8)
                ph = pss.tile([128, D], f32, tag="pss", name=f"ph1_{k}")
                nc.tensor.matmul(
                    ph[:], xg_t[g][:, b, :, :], w1s[:], start=True, stop=True,
                    perf_mode=PM.DoubleRow,
                )
                # w1 was pre-scaled by 16; cast back on the way to fp16
                nc.vector.tensor_scalar(
                    hs1[:, k, :], ph[:], 1.0 / 16.0, 0.0, op0=OP.mult, op1=OP.add
                )

            def elu_half(ps, dst, hh2):
                for n, (o, w) in enumerate(zip(CO, CW)):
                    neg = tmp.tile([128, 512], f32, tag="neg", name=f"neg{n}")
                    nc.vector.tensor_scalar_min(neg[:, :w], ps[n][:], 0.0)
                    ex = tmp.tile([128, 512], f32, tag="ex", name=f"ex{n}")
                    nc.scalar.activation(ex[:, :w], neg[:, :w], AF.Exp)
                    pm1 = tmp.tile([128, 512], f32, tag="pm1", name=f"pm1{n}")
                    nc.vector.tensor_scalar(
                        pm1[:, :w], ps[n][:], 0.0, -1.0, op0=OP.max, op1=OP.add
                    )
                    nc.vector.tensor_add(
                        dst[:, hh2 * sh + o : hh2 * sh + o + w], ex[:, :w], pm1[:, :w]
                    )

            # ================= layer 1 (+ h1 pipelined one tile ahead) =====
            x2t = xepool.tile([128, s], f32, tag="xe", name="x2t")
            for hh2 in (0, 1):
                ps = [
                    psb.tile([128, w], f32, tag=f"bg{n}", name=f"ps1_{hh2}_{n}")
                    for n, w in enumerate(CW)
                ]
                if hh2 == 0:
                    load_xg(0)
                    load_xg(1)
                    for b in range(8):
                        h1_block(b)
                for g in range(NT):
                    at = stream_tile(hh2, g)
                    if hh2 == 0:
                        load_xg(g + 2)
                        if g + 1 < NXG:
                            for b in range(8):
                                h1_block((g + 1) * 8 + b)
                    for j in range(KG):
                        for i in (0, 1):
                            k = g * KG * 2 + j * 2 + i
                            for n, (o, w) in enumerate(zip(CO, CW)):
                                nc.tensor.matmul(
                                    ps[n][:],
                                    hs1[:, k, :],
                                    at[:, j, i, o : o + w],
                                    start=(k == 0),
                                    stop=(k == NB - 1),
                                )
                # ---- boundary hh2: elu, h2, fp8 node-major, AllGather ----
                elu_half(ps, x2t, hh2)
                x2h = x2hp.tile([128, sh], f16, tag="x2h", name=f"x2h{hh2}")
                nc.vector.tensor_copy(x2h[:], x2t[:, hh2 * sh : (hh2 + 1) * sh])
                hT = h2sp.tile([128, sh], f8, tag="h2sT", name=f"h2sT{hh2}")
                for cb in range(RH):
                    cs = slice(cb * 128, (cb + 1) * 128)
                    ph2 = pss.tile([128, D], f32, tag="pss", name=f"ph2_{hh2}_{cb}")
                    nc.tensor.matmul(ph2[:], w2s[:], x2h[:, cs], start=True, stop=True)
                    nc.vector.tensor_copy(hT[:, cs], ph2[:])
                # transpose own blocks to node-major BEFORE the AllGather
                # (fp8 transpose writes 16-bit lanes -> step-2 output view)
                h2n = h2np.tile([128, BS, D], f8, tag="h2n", name=f"h2n{hh2}")
                h2n_t[hh2] = h2n
                for b in range(BS):
                    pt = pss.tile([128, 128, 2], f8, tag="pss", name=f"ptl_{hh2}_{b}")
                    nc.tensor.transpose(
                        pt[:, :, 0], hT[:, b * 128 : (b + 1) * 128], ident[:]
                    )
                    nc.vector.tensor_copy(h2n[:, b, :], pt[:, :, 0])
                ringC.dma_start(h2b[hh2][:], h2n.rearrange("p b d -> p (b d)"))
                nc.gpsimd.collective_compute(
                    "AllGather",
                    OP.bypass,
                    ins=[h2b[hh2].opt()],
                    outs=[h2f[hh2].opt()],
                    replica_groups=rg,
                )
                # remote fetches (node-major -> used directly as stationaries)
                for g in range(ncores - 1):
                    with ringC.register(f"hoff_{hh2}_{g}") as hreg:
                        ringC.reg_load(hreg, hoffs[0:1, g : g + 1])
                        off = ringC.snap(hreg, min_val=0, max_val=(ncores - 1) * 128)
                    hb = hblkp.tile([128, BS, D], f8, tag="hblk",
                                    name=f"hblk{hh2}_{g}")
                    ringC.dma_start(
                        hb.rearrange("p b d -> p (b d)"),
                        h2f[hh2][bass.ds(off, 128), :],
                    )
                    remote_bufs[(hh2, g)] = hb

            def lhsT_pair(kp):
                kk = 2 * kp
                hh, off = divmod(kk, NBH)
                rr, b = divmod(off, BS)
                src = h2n_t[hh] if rr == 0 else remote_bufs[(hh, rr - 1)]
                return src[:, b : b + 2, :]

            # ================= layer 2 (fp8 DoubleRow) ====================
            x3t = xepool.tile([128, s], f32, tag="xe", name="x3t")

            def out_stage(hh2):
                outr = out.ap().rearrange("(c p) m -> c p m", p=128)
                zbig = outp.tile([128, RH, C], f32, tag="zbig", name=f"zbig{hh2}")
                for cb in range(RH):
                    py = psy.tile([128, C], f32, tag="psy", name=f"py{hh2}_{cb}")
                    cs = slice(hh2 * sh + cb * 128, hh2 * sh + (cb + 1) * 128)
                    nc.tensor.matmul(py[:], x3t[:, cs], wouts[:], start=True, stop=True)
                    nc.vector.tensor_add(zbig[:, cb, :], py[:], bbs[:])
                zf = zbig.rearrange("p c m -> p (c m)")
                negb = tmp.tile([128, RH * C], f32, tag="negB", name=f"negb{hh2}")
                nc.vector.tensor_scalar_min(negb[:], zf, 0.0)
                eb = tmp.tile([128, RH * C], f32, tag="exB", name=f"eb{hh2}")
                nc.scalar.activation(eb[:], negb[:], AF.Exp)
                pmb = tmp.tile([128, RH * C], f32, tag="pmB", name=f"pmb{hh2}")
                nc.vector.tensor_scalar(pmb[:], zf, 0.0, -1.0, op0=OP.max, op1=OP.add)
                zzb = outp.tile([128, RH, C], f32, tag="zzb", name=f"zzb{hh2}")
                nc.vector.tensor_add(zzb.rearrange("p c m -> p (c m)"), eb[:], pmb[:])
                negm = stat.tile([128, RH], f32, tag="negm", name=f"negm{hh2}")
                nc.vector.tensor_reduce(
                    negm[:], zzb[:], axis=mybir.AxisListType.X, op=OP.max, negate=True
                )
                ssum = stat.tile([128, RH], f32, tag="ssum", name=f"ssum{hh2}")
                es = tmp.tile([128, RH * C], f32, tag="negB", name=f"es{hh2}")
                esv = es.rearrange("p (c m) -> p c m", m=C)
                for cb in range(RH):
                    nc.scalar.activation(
                        esv[:, cb, :],
                        zzb[:, cb, :],
                        AF.Exp,
                        bias=negm[:, cb : cb + 1],
                        accum_out=ssum[:, cb : cb + 1],
                    )
                lse = stat.tile([128, RH], f32, tag="lse", name=f"lse{hh2}")
                nc.scalar.activation(lse[:], ssum[:], AF.Ln)
                for cb in range(RH):
                    osb = outp.tile([128, C], f32, tag="osb", name=f"osb{hh2}_{cb}")
                    nc.vector.tensor_scalar(
                        osb[:],
                        zzb[:, cb, :],
                        negm[:, cb : cb + 1],
                        lse[:, cb : cb + 1],
                        op0=OP.add,
                        op1=OP.subtract,
                    )
                    # stores on the HWDGE rings (SWDGE serializes ~1.7us each)
                    (ringA if cb % 2 == 0 else ringB).dma_start(
                        outr[hh2 * RH + cb], osb[:]
                    )

            # both output-half phases accumulate concurrently; every tile whose
            # stationaries are already available streams first, maximizing the
            # runway that hides the second AllGather.
            ps2 = {
                hh2: [
                    psb.tile([128, w], f32, tag=f"bg{n}", name=f"ps2_{hh2}_{n}")
                    for n, w in enumerate(CW)
                ]
                for hh2 in (0, 1)
            }
            segs = [(0, 0, GSTAR), (1, 0, GSTAR), (0, GSTAR, NT), (1, GSTAR, NT)]
            for si, (hh2, g0, g1) in enumerate(segs):
                for g in range(g0, g1):
                    at = stream_tile(hh2, g)
                    for j in range(KG):
                        kp = g * KG + j
                        for n, (o, w) in enumerate(zip(CO, CW)):
                            nc.tensor.matmul(
                                ps2[hh2][n][:],
                                lhsT_pair(kp),
                                at[:, j, :, o : o + w],
                                start=(kp == 0),
                                stop=(kp == NB // 2 - 1),
                                perf_mode=PM.DoubleRow,
                            )
                if si == 2:
                    elu_half(ps2[0], x3t, 0)
                    out_stage(0)
                elif si == 3:
                    elu_half(ps2[1], x3t, 1)
                    out_stage(1)

    nc.compile()
    return nc


def make_in_maps(x, adj, W1, W2, Wout, bout, ncores=NCORES):
    import ml_dtypes

    E8 = ml_dtypes.float8_e4m3  # TRN fp8e4 (IEEE-ish, max +-240)
    n_total = adj.shape[0]
    s = n_total // ncores
    sh = s // 2
    NB = n_total // 128
    KG = 4
    NT = NB // (KG * 2)
    NXG = NB // 8
    f, d = W1.shape[1], W1.shape[0] * W1.shape[2]
    w1f = np.ascontiguousarray(
        (W1.transpose(1, 0, 2).reshape(f, d) * 16.0).astype(E8)
    )
    w2f = np.ascontiguousarray(W2.transpose(1, 0, 2).reshape(d, d).astype(np.float16))
    woutf = np.ascontiguousarray(Wout.astype(np.float32))
    bbf = np.ascontiguousarray(
        np.broadcast_to(bout.astype(np.float32), (128, Wout.shape[1]))
    )
    adj8 = adj.astype(E8)
    x8 = x.astype(E8)
    in_maps = []
    for c in range(ncores):
        rows = slice(c * s, (c + 1) * s)
        # contraction permutation: [half 0 | half 1] x [rank c, c+1, ..]
        perm = np.concatenate(
            [
                np.arange(((c + rr) % ncores) * s + hh * sh,
                          ((c + rr) % ncores) * s + (hh + 1) * sh)
                for hh in (0, 1)
                for rr in range(ncores)
            ]
        )
        adjtc = adj8[rows][:, perm].T  # [n_total (perm), s]
        halves = []
        for hh2 in (0, 1):
            Ah = adjtc[:, hh2 * sh : (hh2 + 1) * sh]
            halves.append(
                Ah.reshape(NT, KG, 2, 128, sh)
                .transpose(0, 3, 1, 2, 4)
                .reshape(NT * 128, KG * 2 * sh)
            )
        adjt_np = np.ascontiguousarray(np.concatenate(halves, axis=0))
        # xc[g*128 + p, ((j*2 + a)*128) + m] = xperm.T[a*128 + p, (g*8 + j)*128 + m]
        xtc = x8[perm].T  # [F, n_total]
        xcf = np.ascontiguousarray(
            xtc.reshape(2, 128, NXG, 8, 128)
            .transpose(2, 1, 3, 0, 4)
            .reshape(NXG * 128, 8 * f)
        )
        hoffc = np.zeros((1, 8), np.uint32)
        for g in range(ncores - 1):
            hoffc[0, g] = ((c + 1 + g) % ncores) * 128
        in_maps.append(
            {
                "adjt": adjt_np,
                "xc": xcf,
                "w1": w1f,
                "w2": w2f,
                "wout": woutf,
                "bb": bbf,
                "hoff": hoffc,
            }
        )
    return in_maps


def kernel(x, adj, W1, W2, Wout, bout):
    from concourse import bass_utils

    x = np.asarray(x)
    adj = np.asarray(adj)
    in_maps = make_in_maps(x, adj, np.asarray(W1), np.asarray(W2),
                           np.asarray(Wout), np.asarray(bout))
    if "nc" not in _nc_cache:
        _nc_cache["nc"] = build_gat_nc()
    res = bass_utils.run_bass_kernel_spmd(
        _nc_cache["nc"], in_maps, core_ids=list(range(NCORES))
    )
    return np.concatenate([r["out"] for r in res.results], axis=0).astype(np.float32)


# revision 15
# speedup vs baseline: 1.7388x; 1.0001x over previous
"""Bass/Trainium2 kernel for the (dead-attention) GAT reference.

Effective math (see reference):
    h1  = x @ W1f                 W1f = W1.transpose(1,0,2).reshape(256,128)
    hp1 = elu(adj @ h1)
    h2  = hp1 @ W2f               W2f = W2.transpose(1,0,2).reshape(128,128)
    hp2 = elu(adj @ h2)
    y   = elu(hp2 @ Wout + bout)
    out = log_softmax(y, axis=1)

Distribution + precision strategy:
  * adj row-sharded 8 ways; each core's 2048x16384 shard is uploaded
    TRANSPOSED, fp8(e4m3), pre-tiled for big contiguous DMA lines, with
    contraction rows ordered [all ranks' first node-half | second half]
    so all layer-2 stationaries that depend on the final AllGather are
    consumed last.  Every adj tile is split across BOTH hardware DMA
    rings so a briefly-blocked ring never strands half the stream.
  * h1 is computed replicated from an fp8 x stream, one DoubleRow
    matmul per 128-node block (W1 pre-scaled by 16 to dodge fp8
    subnormals; the PSUM->SBUF cast divides it back).  The h1
    STATIONARY stays fp16: layer-1 h precision dominates final error
    (it is amplified by both all-positive adj matmuls).  h1 for tile
    g+1 is emitted ahead of tile g's adj matmuls so the PE never waits
    on the PSUM->SBUF cast.
  * Layer 2 uses fp8 h2 x fp8 adj with DoubleRow.  h2 is exchanged
    NODE-major (own blocks PE-transposed before the fp8 AllGather); the
    gathered buffer is fetched with plain static DMAs (rank-piece
    granularity for the second half) and used directly as matmul
    stationaries - no unpack transposes, no register-indexed DMAs.
  * Each layer is split into two output-column halves: the half-0 h2
    AllGather flies under layer 1's half-1 stream; layer 2 first
    consumes every first-half contraction pair of BOTH output halves
    (~35us runway) to cover the second AllGather, and half 0's output
    stage overlaps half 1's stream.
"""

import sys

import numpy as np

sys.path.insert(0, "/opt/trn_rl_repo")

N = 16384  # nodes
F = 256  # input features
D = 128  # hidden width (nheads*nhid)
C = 32  # classes
NCORES = 8
S = N // NCORES  # rows per core

_nc_cache = {}


def build_gat_nc(n_total=N, ncores=NCORES, enable_asserts=False, abufs=13):
    """Build the SPMD Bass program (one program, runs on all cores)."""
    from contextlib import ExitStack

    from concourse import bacc, bass, masks, mybir, tile

    s = n_total // ncores  # shard rows per core
    sh = s // 2  # output columns per half
    NB = n_total // 128  # contraction blocks
    NBH = NB // 2  # blocks per contraction-half
    BS = NBH // ncores  # blocks per rank segment (per half)
    assert BS % 2 == 0, "pairs must not straddle rank segments"
    KG = 4  # contraction pairs per adj tile
    NT = NB // (KG * 2)  # adj tiles per output-half phase
    NXG = NB // 8  # xc groups (8 blocks each)
    RH = sh // 128  # 128-node output chunks per half
    CW = [min(512, sh - o) for o in range(0, sh, 512)]
    CO = list(range(0, sh, 512))
    GSTAR = NBH // (KG * 2)  # first L2 tile with second-half pairs
    f32 = mybir.dt.float32
    f16 = mybir.dt.float16
    f8 = mybir.dt.float8e4
    AF = mybir.ActivationFunctionType
    OP = mybir.AluOpType
    PM = mybir.MatmulPerfMode

    nc = bacc.Bacc(
        "TRN2",
        target_bir_lowering=False,
        debug=False,
        enable_asserts=enable_asserts,
        num_devices=ncores,
    )

    adjt = nc.dram_tensor("adjt", [2 * NT * 128, KG * 2 * sh], f8, kind="ExternalInput")
    xc = nc.dram_tensor("xc", [NXG * 128, 8 * F], f8, kind="ExternalInput")
    w1 = nc.dram_tensor("w1", [F, D], f8, kind="ExternalInput")  # pre-scaled x16
    w2 = nc.dram_tensor("w2", [D, D], f16, kind="ExternalInput")
    wout = nc.dram_tensor("wout", [D, C], f32, kind="ExternalInput")
    bb = nc.dram_tensor("bb", [128, (sh // 128) * C], f32, kind="ExternalInput")
    out = nc.dram_tensor("out", [s, C], f32, kind="ExternalOutput")

    rg = [list(range(ncores))]

    with ExitStack() as stack:
        tc = stack.enter_context(tile.TileContext(nc))
        pool = lambda **kw: stack.enter_context(tc.tile_pool(**kw))
        dram = pool(name="dram", bufs=1, space="DRAM")
        const = pool(name="const", bufs=1)
        hs1p = pool(name="hs1p", bufs=1)
        hfp = pool(name="hfp", bufs=2)
        apool = pool(name="adjs", bufs=abufs)
        xcpool = pool(name="xcp", bufs=3)
        xepool = pool(name="xe", bufs=1)
        x2hp = pool(name="x2hp", bufs=2)
        h2sp = pool(name="h2sp", bufs=2)
        h2np = pool(name="h2np", bufs=2)
        tmp = pool(name="tmp", bufs=2)
        outp = pool(name="outp", bufs=4)
        stat = pool(name="stat", bufs=2)
        psb = pool(name="psb", bufs=2, space="PSUM")
        pss = pool(name="pss", bufs=3, space="PSUM")
        psy = pool(name="psy", bufs=1, space="PSUM")
        if True:
            ringA, ringB, ringC = nc.sync, nc.scalar, nc.gpsimd

            # --- replicated constants (SWDGE so HW rings start streaming) ---
            w1s = const.tile([128, 2, D], f8, tag="w1s")
            ringC.dma_start(w1s[:], w1.ap().rearrange("(a p) m -> p a m", p=128))
            w2s = const.tile([128, D], f16, tag="w2s")
            ringC.dma_start(w2s[:], w2.ap())
            wouts = const.tile([128, C], f32, tag="wouts")
            ringC.dma_start(wouts[:], wout.ap())
            bbs = const.tile([128, RH * C], f32, tag="bbs")
            ringC.dma_start(bbs[:], bb.ap())
            ident = const.tile([128, 128], f8, tag="ident")
            masks.make_identity(nc, ident[:])

            # --- DRAM bounce buffers for the two half AllGathers (fp8) ---
            h2b = [dram.tile([128, BS * D], f8, tag="h2b", name=f"h2b{h}")
                   for h in (0, 1)]
            h2f = [
                dram.tile([128 * ncores, BS * D], f8, tag="h2f", name=f"h2f{h}",
                          addr_space="Shared")
                for h in (0, 1)
            ]
            # gathered node-major h2, used directly as layer-2 stationaries
            hfull = [
                hfp.tile([128, ncores, BS, D], f8, tag="hfull", name=f"hfull{h}")
                for h in (0, 1)
            ]

            ar = adjt.ap().rearrange("(t p) m -> t p m", p=128)
            xr = xc.ap().rearrange("(g p) q -> g p q", p=128)
            h2n_t = [None, None]

            hs1 = hs1p.tile([128, NB, D], f16, tag="hs1")

            def stream_tile(hh2, g):
                # split every tile across BOTH rings: a stalled ring then
                # never strands half of the remaining stream
                at = apool.tile([128, KG, 2, sh], f8, tag="adj")
                t = hh2 * NT + g
                half = KG // 2
                ringA.dma_start(
                    at[:, :half, :, :].rearrange("p j i m -> p (j i m)"),
                    ar[t][:, : KG * sh],
                )
                ringB.dma_start(
                    at[:, half:, :, :].rearrange("p j i m -> p (j i m)"),
                    ar[t][:, KG * sh :],
                )
                return at

            xg_t = {}

            def load_xg(g):
                if 0 <= g < NXG:
                    t = xcpool.tile([128, 8, 2, 128], f8, tag="xg", name=f"xg{g}")
                    (ringB if g % 2 == 0 else ringA).dma_start(
                        t.rearrange("p j a m -> p (j a m)"), xr[g]
                    )
                    xg_t[g] = t

            def h1_block(k):
                # one DoubleRow matmul: both 128-feature halves in one pass
                g, b = divmod(k, 8)
                ph = pss.tile([128, D], f32, tag="pss", name=f"ph1_{k}")
                nc.tensor.matmul(
                    ph[:], xg_t[g][:, b, :, :], w1s[:], start=True, stop=True,
                    perf_mode=PM.DoubleRow,
                )
                # w1 was pre-scaled by 16; cast back on the way to fp16
                nc.vector.tensor_scalar(
                    hs1[:, k, :], ph[:], 1.0 / 16.0, 0.0, op0=OP.mult, op1=OP.add
                )

            def elu_half(ps, dst, off):
                # dst[:, off:off+sh] = elu(psum chunks); dst dtype may be f16
                for n, (o, w) in enumerate(zip(CO, CW)):
                    neg = tmp.tile([128, 512], f32, tag="neg", name=f"neg{n}")
                    nc.vector.tensor_scalar_min(neg[:, :w], ps[n][:], 0.0)
                    ex = tmp.tile([128, 512], f32, tag="ex", name=f"ex{n}")
                    nc.scalar.activation(ex[:, :w], neg[:, :w], AF.Exp)
                    pm1 = tmp.tile([128, 512], f32, tag="pm1", name=f"pm1{n}")
                    nc.vector.tensor_scalar(
                        pm1[:, :w], ps[n][:], 0.0, -1.0, op0=OP.max, op1=OP.add
                    )
                    nc.vector.tensor_add(
                        dst[:, off + o : off + o + w], ex[:, :w], pm1[:, :w]
                    )

            # ================= layer 1 (+ h1 pipelined one tile ahead) =====
            for hh2 in (0, 1):
                ps = [
                    psb.tile([128, w], f32, tag=f"bg{n}", name=f"ps1_{hh2}_{n}")
                    for n, w in enumerate(CW)
                ]
                if hh2 == 0:
                    load_xg(0)
                    load_xg(1)
                    for b in range(8):
                        h1_block(b)
                for g in range(NT):
                    at = stream_tile(hh2, g)
                    if hh2 == 0:
                        load_xg(g + 2)
                        if g + 1 < NXG:
                            for b in range(8):
                                h1_block((g + 1) * 8 + b)
                    for j in range(KG):
                        for i in (0, 1):
                            k = g * KG * 2 + j * 2 + i
                            for n, (o, w) in enumerate(zip(CO, CW)):
                                nc.tensor.matmul(
                                    ps[n][:],
                                    hs1[:, k, :],
                                    at[:, j, i, o : o + w],
                                    start=(k == 0),
                                    stop=(k == NB - 1),
                                )
                # ---- boundary hh2: elu -> f16, h2, fp8 node-major, AG ----
                x2h = x2hp.tile([128, sh], f16, tag="x2h", name=f"x2h{hh2}")
                elu_half(ps, x2h, 0)
                hT = h2sp.tile([128, sh], f8, tag="h2sT", name=f"h2sT{hh2}")
                for cb in range(RH):
                    cs = slice(cb * 128, (cb + 1) * 128)
                    ph2 = pss.tile([128, D], f32, tag="pss", name=f"ph2_{hh2}_{cb}")
                    nc.tensor.matmul(ph2[:], w2s[:], x2h[:, cs], start=True, stop=True)
                    nc.vector.tensor_copy(hT[:, cs], ph2[:])
                # transpose own blocks to node-major BEFORE the AllGather
                # (fp8 transpose writes 16-bit lanes -> step-2 output view)
                h2n = h2np.tile([128, BS, D], f8, tag="h2n", name=f"h2n{hh2}")
                h2n_t[hh2] = h2n
                for b in range(BS):
                    pt = pss.tile([128, 128, 2], f8, tag="pss", name=f"ptl_{hh2}_{b}")
                    nc.tensor.transpose(
                        pt[:, :, 0], hT[:, b * 128 : (b + 1) * 128], ident[:]
                    )
                    nc.vector.tensor_copy(h2n[:, b, :], pt[:, :, 0])
                ringC.dma_start(h2b[hh2][:], h2n.rearrange("p b d -> p (b d)"))
                nc.gpsimd.collective_compute(
                    "AllGather",
                    OP.bypass,
                    ins=[h2b[hh2].opt()],
                    outs=[h2f[hh2].opt()],
                    replica_groups=rg,
                )
                if hh2 == 0:
                    # whole gathered first half in one static SWDGE fetch
                    ringC.dma_start(
                        hfull[0].rearrange("p r b d -> p (r b d)"),
                        h2f[0].rearrange("(r p) m -> p r m", p=128),
                    )

            def lhsT_pair(kp):
                kk = 2 * kp
                hh, off = divmod(kk, NBH)
                rr, b = divmod(off, BS)
                return hfull[hh][:, rr, b : b + 2, :]

            # ================= layer 2 (fp8 DoubleRow) ====================
            x3t = xepool.tile([128, s], f32, tag="xe", name="x3t")

            def out_stage(hh2):
                outr = out.ap().rearrange("(c p) m -> c p m", p=128)
                pya = psy.tile([128, RH, C], f32, tag="psy", name=f"pya{hh2}")
                for cb in range(RH):
                    cs = slice(hh2 * sh + cb * 128, hh2 * sh + (cb + 1) * 128)
                    nc.tensor.matmul(
                        pya[:, cb, :], x3t[:, cs], wouts[:], start=True, stop=True
                    )
                zbig = outp.tile([128, RH, C], f32, tag="zbig", name=f"zbig{hh2}")
                nc.vector.tensor_add(
                    zbig.rearrange("p c m -> p (c m)"),
                    pya.rearrange("p c m -> p (c m)"),
                    bbs[:],
                )
                zf = zbig.rearrange("p c m -> p (c m)")
                negb = tmp.tile([128, RH * C], f32, tag="negB", name=f"negb{hh2}")
                nc.vector.tensor_scalar_min(negb[:], zf, 0.0)
                eb = tmp.tile([128, RH * C], f32, tag="exB", name=f"eb{hh2}")
                nc.scalar.activation(eb[:], negb[:], AF.Exp)
                pmb = tmp.tile([128, RH * C], f32, tag="pmB", name=f"pmb{hh2}")
                nc.vector.tensor_scalar(pmb[:], zf, 0.0, -1.0, op0=OP.max, op1=OP.add)
                zzb = outp.tile([128, RH, C], f32, tag="zzb", name=f"zzb{hh2}")
                nc.vector.tensor_add(zzb.rearrange("p c m -> p (c m)"), eb[:], pmb[:])
                negm = stat.tile([128, RH], f32, tag="negm", name=f"negm{hh2}")
                nc.vector.tensor_reduce(
                    negm[:], zzb[:], axis=mybir.AxisListType.X, op=OP.max, negate=True
                )
                ssum = stat.tile([128, RH], f32, tag="ssum", name=f"ssum{hh2}")
                es = tmp.tile([128, RH * C], f32, tag="negB", name=f"es{hh2}")
                esv = es.rearrange("p (c m) -> p c m", m=C)
                for cb in range(RH):
                    nc.scalar.activation(
                        esv[:, cb, :],
                        zzb[:, cb, :],
                        AF.Exp,
                        bias=negm[:, cb : cb + 1],
                        accum_out=ssum[:, cb : cb + 1],
                    )
                lse = stat.tile([128, RH], f32, tag="lse", name=f"lse{hh2}")
                nc.scalar.activation(lse[:], ssum[:], AF.Ln)
                for cb in range(RH):
                    osb = outp.tile([128, C], f32, tag="osb", name=f"osb{hh2}_{cb}")
                    nc.vector.tensor_scalar(
                        osb[:],
                        zzb[:, cb, :],
                        negm[:, cb : cb + 1],
                        lse[:, cb : cb + 1],
                        op0=OP.add,
                        op1=OP.subtract,
                    )
                    (ringA if cb % 2 == 0 else ringB).dma_start(
                        outr[hh2 * RH + cb], osb[:]
                    )

            ps2 = {
                hh2: [
                    psb.tile([128, w], f32, tag=f"bg{n}", name=f"ps2_{hh2}_{n}")
                    for n, w in enumerate(CW)
                ]
                for hh2 in (0, 1)
            }
            segs = [(0, 0, GSTAR), (1, 0, GSTAR), (0, GSTAR, NT), (1, GSTAR, NT)]
            for si, (hh2, g0, g1) in enumerate(segs):
                if si == 2:
                    # second gathered half: static rank-piece fetches on the
                    # scalar HW ring, in consumption order
                    for r in range(ncores):
                        ringB.dma_start(
                            hfull[1][:, r, :, :].rearrange("p b d -> p (b d)"),
                            h2f[1][r * 128 : (r + 1) * 128, :],
                        )
                for g in range(g0, g1):
                    at = stream_tile(hh2, g)
                    for j in range(KG):
                        kp = g * KG + j
                        for n, (o, w) in enumerate(zip(CO, CW)):
                            nc.tensor.matmul(
                                ps2[hh2][n][:],
                                lhsT_pair(kp),
                                at[:, j, :, o : o + w],
                                start=(kp == 0),
                                stop=(kp == NB // 2 - 1),
                                perf_mode=PM.DoubleRow,
                            )
                if si == 2:
                    elu_half(ps2[0], x3t, 0)
                    out_stage(0)
                elif si == 3:
                    elu_half(ps2[1], x3t, sh)
                    out_stage(1)

    nc.compile()
    return nc


def make_in_maps(x, adj, W1, W2, Wout, bout, ncores=NCORES):
    import ml_dtypes

    E8 = ml_dtypes.float8_e4m3  # TRN fp8e4 (IEEE-ish, max +-240)
    n_total = adj.shape[0]
    s = n_total // ncores
    sh = s // 2
    NB = n_total // 128
    KG = 4
    NT = NB // (KG * 2)
    NXG = NB // 8
    RH = sh // 128
    f, d = W1.shape[1], W1.shape[0] * W1.shape[2]
    w1f = np.ascontiguousarray(
        (W1.transpose(1, 0, 2).reshape(f, d) * 16.0).astype(E8)
    )
    w2f = np.ascontiguousarray(W2.transpose(1, 0, 2).reshape(d, d).astype(np.float16))
    woutf = np.ascontiguousarray(Wout.astype(np.float32))
    bbf = np.ascontiguousarray(
        np.broadcast_to(
            np.tile(bout.astype(np.float32), RH), (128, RH * Wout.shape[1])
        )
    )
    adj8 = adj.astype(E8)
    x8 = x.astype(E8)
    # global contraction order: [all ranks' half 0 | all ranks' half 1]
    perm = np.concatenate(
        [
            np.arange(r * s + hh * sh, r * s + (hh + 1) * sh)
            for hh in (0, 1)
            for r in range(ncores)
        ]
    )
    xtc = x8[perm].T  # [F, n_total]
    xcf = np.ascontiguousarray(
        xtc.reshape(2, 128, NXG, 8, 128)
        .transpose(2, 1, 3, 0, 4)
        .reshape(NXG * 128, 8 * f)
    )
    in_maps = []
    for c in range(ncores):
        rows = slice(c * s, (c + 1) * s)
        adjtc = adj8[rows][:, perm].T  # [n_total (perm), s]
        halves = []
        for hh2 in (0, 1):
            Ah = adjtc[:, hh2 * sh : (hh2 + 1) * sh]
            halves.append(
                Ah.reshape(NT, KG, 2, 128, sh)
                .transpose(0, 3, 1, 2, 4)
                .reshape(NT * 128, KG * 2 * sh)
            )
        adjt_np = np.ascontiguousarray(np.concatenate(halves, axis=0))
        in_maps.append(
            {
                "adjt": adjt_np,
                "xc": xcf,
                "w1": w1f,
                "w2": w2f,
                "wout": woutf,
                "bb": bbf,
            }
        )
    return in_maps


def kernel(x, adj, W1, W2, Wout, bout):
    from concourse import bass_utils

    x = np.asarray(x)
    adj = np.asarray(adj)
    in_maps = make_in_maps(x, adj, np.asarray(W1), np.asarray(W2),
                           np.asarray(Wout), np.asarray(bout))
    if "nc" not in _nc_cache:
        _nc_cache["nc"] = build_gat_nc()
    res = bass_utils.run_bass_kernel_spmd(
        _nc_cache["nc"], in_maps, core_ids=list(range(NCORES))
    )
    return np.concatenate([r["out"] for r in res.results], axis=0).astype(np.float32)


# revision 18
# speedup vs baseline: 1.8032x; 1.0370x over previous
"""Bass/Trainium2 kernel for the (dead-attention) GAT reference.

Effective math (see reference):
    h1  = x @ W1f                 W1f = W1.transpose(1,0,2).reshape(256,128)
    hp1 = elu(adj @ h1)
    h2  = hp1 @ W2f               W2f = W2.transpose(1,0,2).reshape(128,128)
    hp2 = elu(adj @ h2)
    y   = elu(hp2 @ Wout + bout)
    out = log_softmax(y, axis=1)

Distribution + precision strategy:
  * adj row-sharded 8 ways; each core's 2048x16384 shard is uploaded
    TRANSPOSED, fp8(e4m3), pre-tiled for big contiguous DMA lines, with
    contraction rows ordered [all ranks' first node-half | second half]
    so all layer-2 stationaries that depend on the final AllGather are
    consumed last.  Every adj tile is split across BOTH hardware DMA
    rings so a briefly-blocked ring never strands half the stream.
  * h1 is computed replicated from an fp8 x stream, one DoubleRow
    matmul per 128-node block (W1 pre-scaled by 16 to dodge fp8
    subnormals; the PSUM->SBUF cast divides it back).  The h1
    STATIONARY stays fp16: layer-1 h precision dominates final error
    (it is amplified by both all-positive adj matmuls).  h1 for tile
    g+1 is emitted ahead of tile g's adj matmuls so the PE never waits
    on the PSUM->SBUF cast.
  * Layer 2 uses fp8 h2 x fp8 adj with DoubleRow.  h2 is exchanged
    NODE-major (own blocks PE-transposed before the fp8 AllGather); the
    gathered buffer is fetched with plain static DMAs (rank-piece
    granularity for the second half) and used directly as matmul
    stationaries - no unpack transposes, no register-indexed DMAs.
  * Each layer is split into two output-column halves: the half-0 h2
    AllGather flies under layer 1's half-1 stream; layer 2 first
    consumes every first-half contraction pair of BOTH output halves
    (~35us runway) to cover the second AllGather, and half 0's output
    stage overlaps half 1's stream.
"""

import sys

import numpy as np

sys.path.insert(0, "/opt/trn_rl_repo")

N = 16384  # nodes
F = 256  # input features
D = 128  # hidden width (nheads*nhid)
C = 32  # classes
NCORES = 8
S = N // NCORES  # rows per core

_nc_cache = {}


def build_gat_nc(n_total=N, ncores=NCORES, enable_asserts=False, abufs=13):
    """Build the SPMD Bass program (one program, runs on all cores)."""
    from contextlib import ExitStack

    from concourse import bacc, bass, masks, mybir, tile

    s = n_total // ncores  # shard rows per core
    sh = s // 2  # output columns per half
    NB = n_total // 128  # contraction blocks
    NBH = NB // 2  # blocks per contraction-half
    BS = NBH // ncores  # blocks per rank segment (per half)
    assert BS % 2 == 0, "pairs must not straddle rank segments"
    KG = 4  # contraction pairs per adj tile
    NT = NB // (KG * 2)  # adj tiles per output-half phase
    NXG = NB // 8  # xc groups (8 blocks each)
    RH = sh // 128  # 128-node output chunks per half
    CW = [min(512, sh - o) for o in range(0, sh, 512)]
    CO = list(range(0, sh, 512))
    GSTAR = NBH // (KG * 2)  # first L2 tile with second-half pairs
    f32 = mybir.dt.float32
    f16 = mybir.dt.float16
    f8 = mybir.dt.float8e4
    AF = mybir.ActivationFunctionType
    OP = mybir.AluOpType
    PM = mybir.MatmulPerfMode

    nc = bacc.Bacc(
        "TRN2",
        target_bir_lowering=False,
        debug=False,
        enable_asserts=enable_asserts,
        num_devices=ncores,
    )

    adjt = nc.dram_tensor("adjt", [2 * NT * 128, KG * 2 * sh], f8, kind="ExternalInput")
    xc = nc.dram_tensor("xc", [NXG * 128, 8 * F], f8, kind="ExternalInput")
    w1 = nc.dram_tensor("w1", [F, D], f8, kind="ExternalInput")  # pre-scaled x16
    w2 = nc.dram_tensor("w2", [D, D], f16, kind="ExternalInput")
    wout = nc.dram_tensor("wout", [D, C], f32, kind="ExternalInput")
    bb = nc.dram_tensor("bb", [128, (sh // 128) * C], f32, kind="ExternalInput")
    out = nc.dram_tensor("out", [s, C], f32, kind="ExternalOutput")

    rg = [list(range(ncores))]

    with ExitStack() as stack:
        tc = stack.enter_context(tile.TileContext(nc))
        pool = lambda **kw: stack.enter_context(tc.tile_pool(**kw))
        dram = pool(name="dram", bufs=1, space="DRAM")
        const = pool(name="const", bufs=1)
        hs1p = pool(name="hs1p", bufs=1)
        hfp = pool(name="hfp", bufs=2)
        apool = pool(name="adjs", bufs=abufs)
        xcpool = pool(name="xcp", bufs=3)
        xepool = pool(name="xe", bufs=1)
        x2hp = pool(name="x2hp", bufs=2)
        h2sp = pool(name="h2sp", bufs=2)
        h2np = pool(name="h2np", bufs=2)
        tmp = pool(name="tmp", bufs=2)
        outp = pool(name="outp", bufs=4)
        stat = pool(name="stat", bufs=2)
        psb = pool(name="psb", bufs=2, space="PSUM")
        pss = pool(name="pss", bufs=3, space="PSUM")
        psy = pool(name="psy", bufs=1, space="PSUM")
        if True:
            ringA, ringB, ringC = nc.sync, nc.scalar, nc.gpsimd

            # --- replicated constants (SWDGE so HW rings start streaming) ---
            w1s = const.tile([128, 2, D], f8, tag="w1s")
            ringC.dma_start(w1s[:], w1.ap().rearrange("(a p) m -> p a m", p=128))
            w2s = const.tile([128, D], f16, tag="w2s")
            ringC.dma_start(w2s[:], w2.ap())
            wouts = const.tile([128, C], f32, tag="wouts")
            ringC.dma_start(wouts[:], wout.ap())
            bbs = const.tile([128, RH * C], f32, tag="bbs")
            ringC.dma_start(bbs[:], bb.ap())
            ident = const.tile([128, 128], f8, tag="ident")
            masks.make_identity(nc, ident[:])

            # --- DRAM bounce buffers for the two half AllGathers (fp8) ---
            h2b = [dram.tile([128, BS * D], f8, tag="h2b", name=f"h2b{h}")
                   for h in (0, 1)]
            h2f = [
                dram.tile([128 * ncores, BS * D], f8, tag="h2f", name=f"h2f{h}",
                          addr_space="Shared")
                for h in (0, 1)
            ]
            # gathered node-major h2, used directly as layer-2 stationaries
            hfull = [
                hfp.tile([128, ncores, BS, D], f8, tag="hfull", name=f"hfull{h}")
                for h in (0, 1)
            ]

            ar = adjt.ap().rearrange("(t p) m -> t p m", p=128)
            xr = xc.ap().rearrange("(g p) q -> g p q", p=128)
            h2n_t = [None, None]

            # layer-1 stationaries: even contraction pairs fp16, odd pairs
            # fp8 DoubleRow (empirically 1.2e-2 final rel err vs the 2e-2
            # gate; halves the fp16 LDW+MM cost for half the pairs)
            hs1 = hs1p.tile([128, NB // 4, 2, D], f16, tag="hs1")
            hs18 = hs1p.tile([128, NB // 4, 2, D], f8, tag="hs18")

            def stream_tile(hh2, g):
                # split every tile across BOTH rings: a stalled ring then
                # never strands half of the remaining stream
                at = apool.tile([128, KG, 2, sh], f8, tag="adj")
                t = hh2 * NT + g
                half = KG // 2
                ringA.dma_start(
                    at[:, :half, :, :].rearrange("p j i m -> p (j i m)"),
                    ar[t][:, : KG * sh],
                )
                ringB.dma_start(
                    at[:, half:, :, :].rearrange("p j i m -> p (j i m)"),
                    ar[t][:, KG * sh :],
                )
                return at

            xg_t = {}

            def load_xg(g):
                if 0 <= g < NXG:
                    t = xcpool.tile([128, 8, 2, 128], f8, tag="xg", name=f"xg{g}")
                    (ringB if g % 2 == 0 else ringA).dma_start(
                        t.rearrange("p j a m -> p (j a m)"), xr[g]
                    )
                    xg_t[g] = t

            def h1_block(k):
                # one DoubleRow matmul: both 128-feature halves in one pass
                g, b = divmod(k, 8)
                ph = pss.tile([128, D], f32, tag="pss", name=f"ph1_{k}")
                nc.tensor.matmul(
                    ph[:], xg_t[g][:, b, :, :], w1s[:], start=True, stop=True,
                    perf_mode=PM.DoubleRow,
                )
                # w1 was pre-scaled by 16; cast back on the way out of PSUM
                kp = k // 2
                dst = (hs18 if kp % 2 == 1 else hs1)[:, kp // 2, k % 2, :]
                nc.vector.tensor_scalar(
                    dst, ph[:], 1.0 / 16.0, 0.0, op0=OP.mult, op1=OP.add
                )

            def elu_half(ps, dst, off):
                # dst[:, off:off+sh] = elu(psum chunks); dst dtype may be f16
                for n, (o, w) in enumerate(zip(CO, CW)):
                    neg = tmp.tile([128, 512], f32, tag="neg", name=f"neg{n}")
                    nc.vector.tensor_scalar_min(neg[:, :w], ps[n][:], 0.0)
                    ex = tmp.tile([128, 512], f32, tag="ex", name=f"ex{n}")
                    nc.scalar.activation(ex[:, :w], neg[:, :w], AF.Exp)
                    pm1 = tmp.tile([128, 512], f32, tag="pm1", name=f"pm1{n}")
                    nc.vector.tensor_scalar(
                        pm1[:, :w], ps[n][:], 0.0, -1.0, op0=OP.max, op1=OP.add
                    )
                    nc.vector.tensor_add(
                        dst[:, off + o : off + o + w], ex[:, :w], pm1[:, :w]
                    )

            # ================= layer 1 (+ h1 pipelined one tile ahead) =====
            for hh2 in (0, 1):
                ps = [
                    psb.tile([128, w], f32, tag=f"bg{n}", name=f"ps1_{hh2}_{n}")
                    for n, w in enumerate(CW)
                ]
                if hh2 == 0:
                    load_xg(0)
                    load_xg(1)
                    for b in range(8):
                        h1_block(b)
                for g in range(NT):
                    at = stream_tile(hh2, g)
                    if hh2 == 0:
                        load_xg(g + 2)
                        if g + 1 < NXG:
                            for b in range(8):
                                h1_block((g + 1) * 8 + b)
                    for j in range(KG):
                        kp = g * KG + j
                        if kp % 2 == 1:
                            # odd pair: one fp8 DoubleRow pass for both blocks
                            for n, (o, w) in enumerate(zip(CO, CW)):
                                nc.tensor.matmul(
                                    ps[n][:],
                                    hs18[:, kp // 2, :, :],
                                    at[:, j, :, o : o + w],
                                    start=False,
                                    stop=(kp == NB // 2 - 1),
                                    perf_mode=PM.DoubleRow,
                                )
                        else:
                            for i in (0, 1):
                                k = kp * 2 + i
                                for n, (o, w) in enumerate(zip(CO, CW)):
                                    nc.tensor.matmul(
                                        ps[n][:],
                                        hs1[:, kp // 2, i, :],
                                        at[:, j, i, o : o + w],
                                        start=(k == 0),
                                        stop=False,
                                    )
                # ---- boundary hh2: elu -> f16, h2, fp8 node-major, AG ----
                x2h = x2hp.tile([128, sh], f16, tag="x2h", name=f"x2h{hh2}")
                elu_half(ps, x2h, 0)
                hT = h2sp.tile([128, sh], f8, tag="h2sT", name=f"h2sT{hh2}")
                for cb in range(RH):
                    cs = slice(cb * 128, (cb + 1) * 128)
                    ph2 = pss.tile([128, D], f32, tag="pss", name=f"ph2_{hh2}_{cb}")
                    nc.tensor.matmul(ph2[:], w2s[:], x2h[:, cs], start=True, stop=True)
                    nc.vector.tensor_copy(hT[:, cs], ph2[:])
                # transpose own blocks to node-major BEFORE the AllGather
                # (fp8 transpose writes 16-bit lanes -> step-2 output view)
                h2n = h2np.tile([128, BS, D], f8, tag="h2n", name=f"h2n{hh2}")
                h2n_t[hh2] = h2n
                for b in range(BS):
                    pt = pss.tile([128, 128, 2], f8, tag="pss", name=f"ptl_{hh2}_{b}")
                    nc.tensor.transpose(
                        pt[:, :, 0], hT[:, b * 128 : (b + 1) * 128], ident[:]
                    )
                    nc.vector.tensor_copy(h2n[:, b, :], pt[:, :, 0])
                ringC.dma_start(h2b[hh2][:], h2n.rearrange("p b d -> p (b d)"))
                nc.gpsimd.collective_compute(
                    "AllGather",
                    OP.bypass,
                    ins=[h2b[hh2].opt()],
                    outs=[h2f[hh2].opt()],
                    replica_groups=rg,
                )
                if hh2 == 0:
                    # whole gathered first half in one static SWDGE fetch
                    ringC.dma_start(
                        hfull[0].rearrange("p r b d -> p (r b d)"),
                        h2f[0].rearrange("(r p) m -> p r m", p=128),
                    )

            def lhsT_pair(kp):
                kk = 2 * kp
                hh, off = divmod(kk, NBH)
                rr, b = divmod(off, BS)
                return hfull[hh][:, rr, b : b + 2, :]

            # ================= layer 2 (fp8 DoubleRow) ====================
            x3t = xepool.tile([128, s], f32, tag="xe", name="x3t")

            def out_stage(hh2):
                outr = out.ap().rearrange("(c p) m -> c p m", p=128)
                pya = psy.tile([128, RH, C], f32, tag="psy", name=f"pya{hh2}")
                for cb in range(RH):
                    cs = slice(hh2 * sh + cb * 128, hh2 * sh + (cb + 1) * 128)
                    nc.tensor.matmul(
                        pya[:, cb, :], x3t[:, cs], wouts[:], start=True, stop=True
                    )
                zbig = outp.tile([128, RH, C], f32, tag="zbig", name=f"zbig{hh2}")
                nc.vector.tensor_add(
                    zbig.rearrange("p c m -> p (c m)"),
                    pya.rearrange("p c m -> p (c m)"),
                    bbs[:],
                )
                zf = zbig.rearrange("p c m -> p (c m)")
                negb = tmp.tile([128, RH * C], f32, tag="negB", name=f"negb{hh2}")
                nc.vector.tensor_scalar_min(negb[:], zf, 0.0)
                eb = tmp.tile([128, RH * C], f32, tag="exB", name=f"eb{hh2}")
                nc.scalar.activation(eb[:], negb[:], AF.Exp)
                pmb = tmp.tile([128, RH * C], f32, tag="pmB", name=f"pmb{hh2}")
                nc.vector.tensor_scalar(pmb[:], zf, 0.0, -1.0, op0=OP.max, op1=OP.add)
                zzb = outp.tile([128, RH, C], f32, tag="zzb", name=f"zzb{hh2}")
                nc.vector.tensor_add(zzb.rearrange("p c m -> p (c m)"), eb[:], pmb[:])
                negm = stat.tile([128, RH], f32, tag="negm", name=f"negm{hh2}")
                nc.vector.tensor_reduce(
                    negm[:], zzb[:], axis=mybir.AxisListType.X, op=OP.max, negate=True
                )
                ssum = stat.tile([128, RH], f32, tag="ssum", name=f"ssum{hh2}")
                es = tmp.tile([128, RH * C], f32, tag="negB", name=f"es{hh2}")
                esv = es.rearrange("p (c m) -> p c m", m=C)
                for cb in range(RH):
                    nc.scalar.activation(
                        esv[:, cb, :],
                        zzb[:, cb, :],
                        AF.Exp,
                        bias=negm[:, cb : cb + 1],
                        accum_out=ssum[:, cb : cb + 1],
                    )
                lse = stat.tile([128, RH], f32, tag="lse", name=f"lse{hh2}")
                nc.scalar.activation(lse[:], ssum[:], AF.Ln)
                for cb in range(RH):
                    osb = outp.tile([128, C], f32, tag="osb", name=f"osb{hh2}_{cb}")
                    nc.vector.tensor_scalar(
                        osb[:],
                        zzb[:, cb, :],
                        negm[:, cb : cb + 1],
                        lse[:, cb : cb + 1],
                        op0=OP.add,
                        op1=OP.subtract,
                    )
                    (ringA if cb % 2 == 0 else ringB).dma_start(
                        outr[hh2 * RH + cb], osb[:]
                    )

            ps2 = {
                hh2: [
                    psb.tile([128, w], f32, tag=f"bg{n}", name=f"ps2_{hh2}_{n}")
                    for n, w in enumerate(CW)
                ]
                for hh2 in (0, 1)
            }
            segs = [(0, 0, GSTAR), (1, 0, GSTAR), (0, GSTAR, NT), (1, GSTAR, NT)]
            for si, (hh2, g0, g1) in enumerate(segs):
                if si == 2:
                    # second gathered half: static rank-piece fetches on the
                    # scalar HW ring, in consumption order
                    for r in range(ncores):
                        ringB.dma_start(
                            hfull[1][:, r, :, :].rearrange("p b d -> p (b d)"),
                            h2f[1][r * 128 : (r + 1) * 128, :],
                        )
                for g in range(g0, g1):
                    at = stream_tile(hh2, g)
                    for j in range(KG):
                        kp = g * KG + j
                        for n, (o, w) in enumerate(zip(CO, CW)):
                            nc.tensor.matmul(
                                ps2[hh2][n][:],
                                lhsT_pair(kp),
                                at[:, j, :, o : o + w],
                                start=(kp == 0),
                                stop=(kp == NB // 2 - 1),
                                perf_mode=PM.DoubleRow,
                            )
                if si == 2:
                    elu_half(ps2[0], x3t, 0)
                    out_stage(0)
                elif si == 3:
                    elu_half(ps2[1], x3t, sh)
                    out_stage(1)

    nc.compile()
    return nc


def make_in_maps(x, adj, W1, W2, Wout, bout, ncores=NCORES):
    import ml_dtypes

    E8 = ml_dtypes.float8_e4m3  # TRN fp8e4 (IEEE-ish, max +-240)
    n_total = adj.shape[0]
    s = n_total // ncores
    sh = s // 2
    NB = n_total // 128
    KG = 4
    NT = NB // (KG * 2)
    NXG = NB // 8
    RH = sh // 128
    f, d = W1.shape[1], W1.shape[0] * W1.shape[2]
    w1f = np.ascontiguousarray(
        (W1.transpose(1, 0, 2).reshape(f, d) * 16.0).astype(E8)
    )
    w2f = np.ascontiguousarray(W2.transpose(1, 0, 2).reshape(d, d).astype(np.float16))
    woutf = np.ascontiguousarray(Wout.astype(np.float32))
    bbf = np.ascontiguousarray(
        np.broadcast_to(
            np.tile(bout.astype(np.float32), RH), (128, RH * Wout.shape[1])
        )
    )
    adj8 = adj.astype(E8)
    x8 = x.astype(E8)
    # global contraction order: [all ranks' half 0 | all ranks' half 1]
    perm = np.concatenate(
        [
            np.arange(r * s + hh * sh, r * s + (hh + 1) * sh)
            for hh in (0, 1)
            for r in range(ncores)
        ]
    )
    xtc = x8[perm].T  # [F, n_total]
    xcf = np.ascontiguousarray(
        xtc.reshape(2, 128, NXG, 8, 128)
        .transpose(2, 1, 3, 0, 4)
        .reshape(NXG * 128, 8 * f)
    )
    in_maps = []
    for c in range(ncores):
        rows = slice(c * s, (c + 1) * s)
        adjtc = adj8[rows][:, perm].T  # [n_total (perm), s]
        halves = []
        for hh2 in (0, 1):
            Ah = adjtc[:, hh2 * sh : (hh2 + 1) * sh]
            halves.append(
                Ah.reshape(NT, KG, 2, 128, sh)
                .transpose(0, 3, 1, 2, 4)
                .reshape(NT * 128, KG * 2 * sh)
            )
        adjt_np = np.ascontiguousarray(np.concatenate(halves, axis=0))
        in_maps.append(
            {
                "adjt": adjt_np,
                "xc": xcf,
                "w1": w1f,
                "w2": w2f,
                "wout": woutf,
                "bb": bbf,
            }
        )
    return in_maps


def kernel(x, adj, W1, W2, Wout, bout):
    from concourse import bass_utils

    x = np.asarray(x)
    adj = np.asarray(adj)
    in_maps = make_in_maps(x, adj, np.asarray(W1), np.asarray(W2),
                           np.asarray(Wout), np.asarray(bout))
    if "nc" not in _nc_cache:
        _nc_cache["nc"] = build_gat_nc()
    res = bass_utils.run_bass_kernel_spmd(
        _nc_cache["nc"], in_maps, core_ids=list(range(NCORES))
    )
    return np.concatenate([r["out"] for r in res.results], axis=0).astype(np.float32)


# revision 20
# speedup vs baseline: 1.8142x; 1.0061x over previous
"""Bass/Trainium2 kernel for the (dead-attention) GAT reference.

Effective math (see reference):
    h1  = x @ W1f                 W1f = W1.transpose(1,0,2).reshape(256,128)
    hp1 = elu(adj @ h1)
    h2  = hp1 @ W2f               W2f = W2.transpose(1,0,2).reshape(128,128)
    hp2 = elu(adj @ h2)
    y   = elu(hp2 @ Wout + bout)
    out = log_softmax(y, axis=1)

Distribution + precision strategy:
  * adj row-sharded 8 ways; each core's 2048x16384 shard is uploaded
    TRANSPOSED, fp8(e4m3), pre-tiled for big contiguous DMA lines, with
    contraction rows ordered [all ranks' first node-half | second half]
    so all layer-2 stationaries that depend on the final AllGather are
    consumed last.  Every adj tile is split across BOTH hardware DMA
    rings so a briefly-blocked ring never strands half the stream.
  * h1 is computed replicated from an fp8 x stream, one DoubleRow
    matmul per 128-node block (W1 pre-scaled by 16 to dodge fp8
    subnormals; the PSUM->SBUF cast divides it back).  The h1
    STATIONARY stays fp16: layer-1 h precision dominates final error
    (it is amplified by both all-positive adj matmuls).  h1 for tile
    g+1 is emitted ahead of tile g's adj matmuls so the PE never waits
    on the PSUM->SBUF cast.
  * Layer 2 uses fp8 h2 x fp8 adj with DoubleRow.  h2 is exchanged
    NODE-major (own blocks PE-transposed before the fp8 AllGather); the
    gathered buffer is fetched with plain static DMAs (rank-piece
    granularity for the second half) and used directly as matmul
    stationaries - no unpack transposes, no register-indexed DMAs.
  * Each layer is split into two output-column halves: the half-0 h2
    AllGather flies under layer 1's half-1 stream; layer 2 first
    consumes every first-half contraction pair of BOTH output halves
    (~35us runway) to cover the second AllGather, and half 0's output
    stage overlaps half 1's stream.
"""

import sys

import numpy as np

sys.path.insert(0, "/opt/trn_rl_repo")

N = 16384  # nodes
F = 256  # input features
D = 128  # hidden width (nheads*nhid)
C = 32  # classes
NCORES = 8
S = N // NCORES  # rows per core

_nc_cache = {}


def build_gat_nc(n_total=N, ncores=NCORES, enable_asserts=False, abufs=13):
    """Build the SPMD Bass program (one program, runs on all cores)."""
    from contextlib import ExitStack

    from concourse import bacc, bass, masks, mybir, tile

    s = n_total // ncores  # shard rows per core
    sh = s // 2  # output columns per half
    NB = n_total // 128  # contraction blocks
    NBH = NB // 2  # blocks per contraction-half
    BS = NBH // ncores  # blocks per rank segment (per half)
    assert BS % 2 == 0, "pairs must not straddle rank segments"
    KG = 4  # contraction pairs per adj tile
    NT = NB // (KG * 2)  # adj tiles per output-half phase
    NXG = NB // 8  # xc groups (8 blocks each)
    RH = sh // 128  # 128-node output chunks per half
    CW = [min(512, sh - o) for o in range(0, sh, 512)]
    CO = list(range(0, sh, 512))
    GSTAR = NBH // (KG * 2)  # first L2 tile with second-half pairs
    f32 = mybir.dt.float32
    f16 = mybir.dt.float16
    f8 = mybir.dt.float8e4
    AF = mybir.ActivationFunctionType
    OP = mybir.AluOpType
    PM = mybir.MatmulPerfMode

    nc = bacc.Bacc(
        "TRN2",
        target_bir_lowering=False,
        debug=False,
        enable_asserts=enable_asserts,
        num_devices=ncores,
    )

    adjt = nc.dram_tensor("adjt", [2 * NT * 128, KG * 2 * sh], f8, kind="ExternalInput")
    xc = nc.dram_tensor("xc", [NXG * 128, 8 * F], f8, kind="ExternalInput")
    w1 = nc.dram_tensor("w1", [F, D], f8, kind="ExternalInput")  # pre-scaled x16
    w2 = nc.dram_tensor("w2", [D, D], f16, kind="ExternalInput")
    wout = nc.dram_tensor("wout", [D, C], f32, kind="ExternalInput")
    bb = nc.dram_tensor("bb", [128, (sh // 128) * C], f32, kind="ExternalInput")
    out = nc.dram_tensor("out", [s, C], f32, kind="ExternalOutput")

    rg = [list(range(ncores))]

    with ExitStack() as stack:
        tc = stack.enter_context(tile.TileContext(nc))
        pool = lambda **kw: stack.enter_context(tc.tile_pool(**kw))
        dram = pool(name="dram", bufs=1, space="DRAM")
        const = pool(name="const", bufs=1)
        hs1p = pool(name="hs1p", bufs=1)
        hfp = pool(name="hfp", bufs=2)
        apool = pool(name="adjs", bufs=abufs)
        xcpool = pool(name="xcp", bufs=3)
        xepool = pool(name="xe", bufs=1)
        x2hp = pool(name="x2hp", bufs=2)
        h2sp = pool(name="h2sp", bufs=2)
        h2np = pool(name="h2np", bufs=2)
        tmp = pool(name="tmp", bufs=2)
        outp = pool(name="outp", bufs=4)
        stat = pool(name="stat", bufs=2)
        psb = pool(name="psb", bufs=2, space="PSUM")
        pss = pool(name="pss", bufs=3, space="PSUM")
        psy = pool(name="psy", bufs=1, space="PSUM")
        if True:
            ringA, ringB, ringC = nc.sync, nc.scalar, nc.gpsimd

            # --- replicated constants (SWDGE so HW rings start streaming) ---
            w1s = const.tile([128, 2, D], f8, tag="w1s")
            ringC.dma_start(w1s[:], w1.ap().rearrange("(a p) m -> p a m", p=128))
            w2s = const.tile([128, D], f16, tag="w2s")
            ringC.dma_start(w2s[:], w2.ap())
            wouts = const.tile([128, C], f32, tag="wouts")
            ringC.dma_start(wouts[:], wout.ap())
            bbs = const.tile([128, RH * C], f32, tag="bbs")
            ringC.dma_start(bbs[:], bb.ap())
            ident = const.tile([128, 128], f8, tag="ident")
            masks.make_identity(nc, ident[:])

            # --- DRAM bounce buffers for the two half AllGathers (fp8) ---
            h2b = [dram.tile([128, BS * D], f8, tag="h2b", name=f"h2b{h}")
                   for h in (0, 1)]
            h2f = [
                dram.tile([128 * ncores, BS * D], f8, tag="h2f", name=f"h2f{h}",
                          addr_space="Shared")
                for h in (0, 1)
            ]
            # gathered node-major h2, used directly as layer-2 stationaries
            hfull = [
                hfp.tile([128, ncores, BS, D], f8, tag="hfull", name=f"hfull{h}")
                for h in (0, 1)
            ]

            ar = adjt.ap().rearrange("(t p) m -> t p m", p=128)
            xr = xc.ap().rearrange("(g p) q -> g p q", p=128)
            h2n_t = [None, None]

            # layer-1 stationaries: even contraction pairs fp16, odd pairs
            # fp8 DoubleRow (empirically 1.2e-2 final rel err vs the 2e-2
            # gate; halves the fp16 LDW+MM cost for half the pairs)
            hs1 = hs1p.tile([128, NB // 4, 2, D], f16, tag="hs1")
            hs18 = hs1p.tile([128, NB // 4, 2, D], f8, tag="hs18")

            def stream_tile(hh2, g):
                # split every tile across BOTH rings: a stalled ring then
                # never strands half of the remaining stream
                at = apool.tile([128, KG, 2, sh], f8, tag="adj")
                t = hh2 * NT + g
                half = KG // 2
                ringA.dma_start(
                    at[:, :half, :, :].rearrange("p j i m -> p (j i m)"),
                    ar[t][:, : KG * sh],
                )
                ringB.dma_start(
                    at[:, half:, :, :].rearrange("p j i m -> p (j i m)"),
                    ar[t][:, KG * sh :],
                )
                return at

            xg_t = {}

            def load_xg(g):
                if 0 <= g < NXG:
                    t = xcpool.tile([128, 8, 2, 128], f8, tag="xg", name=f"xg{g}")
                    (ringB if g % 2 == 0 else ringA).dma_start(
                        t.rearrange("p j a m -> p (j a m)"), xr[g]
                    )
                    xg_t[g] = t

            def h1_block(k):
                # one DoubleRow matmul: both 128-feature halves in one pass
                g, b = divmod(k, 8)
                ph = pss.tile([128, D], f32, tag="pss", name=f"ph1_{k}")
                nc.tensor.matmul(
                    ph[:], xg_t[g][:, b, :, :], w1s[:], start=True, stop=True,
                    perf_mode=PM.DoubleRow,
                )
                # w1 was pre-scaled by 16; cast back on the way out of PSUM
                kp = k // 2
                dst = (hs18 if kp % 2 == 1 else hs1)[:, kp // 2, k % 2, :]
                nc.vector.tensor_scalar(
                    dst, ph[:], 1.0 / 16.0, 0.0, op0=OP.mult, op1=OP.add
                )

            def elu_half(ps, dst, off):
                # dst[:, off:off+sh] = elu(psum chunks); dst dtype may be f16
                for n, (o, w) in enumerate(zip(CO, CW)):
                    neg = tmp.tile([128, 512], f32, tag="neg", name=f"neg{n}")
                    nc.vector.tensor_scalar_min(neg[:, :w], ps[n][:], 0.0)
                    ex = tmp.tile([128, 512], f32, tag="ex", name=f"ex{n}")
                    nc.scalar.activation(ex[:, :w], neg[:, :w], AF.Exp)
                    pm1 = tmp.tile([128, 512], f32, tag="pm1", name=f"pm1{n}")
                    nc.vector.tensor_scalar(
                        pm1[:, :w], ps[n][:], 0.0, -1.0, op0=OP.max, op1=OP.add
                    )
                    nc.vector.tensor_add(
                        dst[:, off + o : off + o + w], ex[:, :w], pm1[:, :w]
                    )

            # ================= layer 1 (+ h1 pipelined one tile ahead) =====
            for hh2 in (0, 1):
                ps = [
                    psb.tile([128, w], f32, tag=f"bg{n}", name=f"ps1_{hh2}_{n}")
                    for n, w in enumerate(CW)
                ]
                if hh2 == 0:
                    load_xg(0)
                    load_xg(1)
                    for b in range(8):
                        h1_block(b)
                for g in range(NT):
                    at = stream_tile(hh2, g)
                    if hh2 == 0:
                        load_xg(g + 2)
                        if g + 1 < NXG:
                            for b in range(8):
                                h1_block((g + 1) * 8 + b)
                    for j in range(KG):
                        kp = g * KG + j
                        if kp % 2 == 1:
                            # odd pair: one fp8 DoubleRow pass for both blocks
                            for n, (o, w) in enumerate(zip(CO, CW)):
                                nc.tensor.matmul(
                                    ps[n][:],
                                    hs18[:, kp // 2, :, :],
                                    at[:, j, :, o : o + w],
                                    start=False,
                                    stop=(kp == NB // 2 - 1),
                                    perf_mode=PM.DoubleRow,
                                )
                        else:
                            for i in (0, 1):
                                k = kp * 2 + i
                                for n, (o, w) in enumerate(zip(CO, CW)):
                                    nc.tensor.matmul(
                                        ps[n][:],
                                        hs1[:, kp // 2, i, :],
                                        at[:, j, i, o : o + w],
                                        start=(k == 0),
                                        stop=False,
                                    )
                # ---- boundary hh2: elu -> f16, h2, fp8 node-major, AG ----
                x2h = x2hp.tile([128, sh], f16, tag="x2h", name=f"x2h{hh2}")
                elu_half(ps, x2h, 0)
                hT = h2sp.tile([128, sh], f8, tag="h2sT", name=f"h2sT{hh2}")
                for cb in range(RH):
                    cs = slice(cb * 128, (cb + 1) * 128)
                    ph2 = pss.tile([128, D], f32, tag="pss", name=f"ph2_{hh2}_{cb}")
                    nc.tensor.matmul(ph2[:], w2s[:], x2h[:, cs], start=True, stop=True)
                    nc.vector.tensor_copy(hT[:, cs], ph2[:])
                # transpose own blocks to node-major BEFORE the AllGather
                # (fp8 transpose writes 16-bit lanes -> step-2 output view)
                h2n = h2np.tile([128, BS, D], f8, tag="h2n", name=f"h2n{hh2}")
                h2n_t[hh2] = h2n
                for b in range(BS):
                    pt = pss.tile([128, 128, 2], f8, tag="pss", name=f"ptl_{hh2}_{b}")
                    nc.tensor.transpose(
                        pt[:, :, 0], hT[:, b * 128 : (b + 1) * 128], ident[:]
                    )
                    nc.vector.tensor_copy(h2n[:, b, :], pt[:, :, 0])
                ringC.dma_start(h2b[hh2][:], h2n.rearrange("p b d -> p (b d)"))
                nc.gpsimd.collective_compute(
                    "AllGather",
                    OP.bypass,
                    ins=[h2b[hh2].opt()],
                    outs=[h2f[hh2].opt()],
                    replica_groups=rg,
                )
            # first-half fetch on the scalar HW ring, after the whole L1
            # loop: keeping it off the gpsimd queue lets the second
            # AllGather trigger immediately at layer-1 end
            for r in range(ncores):
                ringB.dma_start(
                    hfull[0][:, r, :, :].rearrange("p b d -> p (b d)"),
                    h2f[0][r * 128 : (r + 1) * 128, :],
                )

            def lhsT_pair(kp):
                kk = 2 * kp
                hh, off = divmod(kk, NBH)
                rr, b = divmod(off, BS)
                return hfull[hh][:, rr, b : b + 2, :]

            # ================= layer 2 (fp8 DoubleRow) ====================
            x3t = xepool.tile([128, s], f32, tag="xe", name="x3t")

            def out_stage(hh2):
                outr = out.ap().rearrange("(c p) m -> c p m", p=128)
                pya = psy.tile([128, RH, C], f32, tag="psy", name=f"pya{hh2}")
                for cb in range(RH):
                    cs = slice(hh2 * sh + cb * 128, hh2 * sh + (cb + 1) * 128)
                    nc.tensor.matmul(
                        pya[:, cb, :], x3t[:, cs], wouts[:], start=True, stop=True
                    )
                zbig = outp.tile([128, RH, C], f32, tag="zbig", name=f"zbig{hh2}")
                nc.vector.tensor_add(
                    zbig.rearrange("p c m -> p (c m)"),
                    pya.rearrange("p c m -> p (c m)"),
                    bbs[:],
                )
                zf = zbig.rearrange("p c m -> p (c m)")
                negb = tmp.tile([128, RH * C], f32, tag="negB", name=f"negb{hh2}")
                nc.vector.tensor_scalar_min(negb[:], zf, 0.0)
                eb = tmp.tile([128, RH * C], f32, tag="exB", name=f"eb{hh2}")
                nc.scalar.activation(eb[:], negb[:], AF.Exp)
                pmb = tmp.tile([128, RH * C], f32, tag="pmB", name=f"pmb{hh2}")
                nc.vector.tensor_scalar(pmb[:], zf, 0.0, -1.0, op0=OP.max, op1=OP.add)
                zzb = outp.tile([128, RH, C], f32, tag="zzb", name=f"zzb{hh2}")
                nc.vector.tensor_add(zzb.rearrange("p c m -> p (c m)"), eb[:], pmb[:])
                negm = stat.tile([128, RH], f32, tag="negm", name=f"negm{hh2}")
                nc.vector.tensor_reduce(
                    negm[:], zzb[:], axis=mybir.AxisListType.X, op=OP.max, negate=True
                )
                ssum = stat.tile([128, RH], f32, tag="ssum", name=f"ssum{hh2}")
                es = tmp.tile([128, RH * C], f32, tag="negB", name=f"es{hh2}")
                esv = es.rearrange("p (c m) -> p c m", m=C)
                for cb in range(RH):
                    nc.scalar.activation(
                        esv[:, cb, :],
                        zzb[:, cb, :],
                        AF.Exp,
                        bias=negm[:, cb : cb + 1],
                        accum_out=ssum[:, cb : cb + 1],
                    )
                lse = stat.tile([128, RH], f32, tag="lse", name=f"lse{hh2}")
                nc.scalar.activation(lse[:], ssum[:], AF.Ln)
                for cb in range(RH):
                    osb = outp.tile([128, C], f32, tag="osb", name=f"osb{hh2}_{cb}")
                    nc.vector.tensor_scalar(
                        osb[:],
                        zzb[:, cb, :],
                        negm[:, cb : cb + 1],
                        lse[:, cb : cb + 1],
                        op0=OP.add,
                        op1=OP.subtract,
                    )
                    (ringA if cb % 2 == 0 else ringB).dma_start(
                        outr[hh2 * RH + cb], osb[:]
                    )

            ps2 = {
                hh2: [
                    psb.tile([128, w], f32, tag=f"bg{n}", name=f"ps2_{hh2}_{n}")
                    for n, w in enumerate(CW)
                ]
                for hh2 in (0, 1)
            }
            segs = [(0, 0, GSTAR), (1, 0, GSTAR), (0, GSTAR, NT), (1, GSTAR, NT)]
            for si, (hh2, g0, g1) in enumerate(segs):
                if si == 2:
                    # second gathered half: static rank-piece fetches on the
                    # scalar HW ring, in consumption order
                    for r in range(ncores):
                        ringB.dma_start(
                            hfull[1][:, r, :, :].rearrange("p b d -> p (b d)"),
                            h2f[1][r * 128 : (r + 1) * 128, :],
                        )
                for g in range(g0, g1):
                    at = stream_tile(hh2, g)
                    for j in range(KG):
                        kp = g * KG + j
                        for n, (o, w) in enumerate(zip(CO, CW)):
                            nc.tensor.matmul(
                                ps2[hh2][n][:],
                                lhsT_pair(kp),
                                at[:, j, :, o : o + w],
                                start=(kp == 0),
                                stop=(kp == NB // 2 - 1),
                                perf_mode=PM.DoubleRow,
                            )
                if si == 2:
                    elu_half(ps2[0], x3t, 0)
                    out_stage(0)
                elif si == 3:
                    elu_half(ps2[1], x3t, sh)
                    out_stage(1)

    nc.compile()
    return nc


def make_in_maps(x, adj, W1, W2, Wout, bout, ncores=NCORES):
    import ml_dtypes

    E8 = ml_dtypes.float8_e4m3  # TRN fp8e4 (IEEE-ish, max +-240)
    n_total = adj.shape[0]
    s = n_total // ncores
    sh = s // 2
    NB = n_total // 128
    KG = 4
    NT = NB // (KG * 2)
    NXG = NB // 8
    RH = sh // 128
    f, d = W1.shape[1], W1.shape[0] * W1.shape[2]
    w1f = np.ascontiguousarray(
        (W1.transpose(1, 0, 2).reshape(f, d) * 16.0).astype(E8)
    )
    w2f = np.ascontiguousarray(W2.transpose(1, 0, 2).reshape(d, d).astype(np.float16))
    woutf = np.ascontiguousarray(Wout.astype(np.float32))
    bbf = np.ascontiguousarray(
        np.broadcast_to(
            np.tile(bout.astype(np.float32), RH), (128, RH * Wout.shape[1])
        )
    )
    adj8 = adj.astype(E8)
    x8 = x.astype(E8)
    # global contraction order: [all ranks' half 0 | all ranks' half 1]
    perm = np.concatenate(
        [
            np.arange(r * s + hh * sh, r * s + (hh + 1) * sh)
            for hh in (0, 1)
            for r in range(ncores)
        ]
    )
    xtc = x8[perm].T  # [F, n_total]
    xcf = np.ascontiguousarray(
        xtc.reshape(2, 128, NXG, 8, 128)
        .transpose(2, 1, 3, 0, 4)
        .reshape(NXG * 128, 8 * f)
    )
    in_maps = []
    for c in range(ncores):
        rows = slice(c * s, (c + 1) * s)
        adjtc = adj8[rows][:, perm].T  # [n_total (perm), s]
        halves = []
        for hh2 in (0, 1):
            Ah = adjtc[:, hh2 * sh : (hh2 + 1) * sh]
            halves.append(
                Ah.reshape(NT, KG, 2, 128, sh)
                .transpose(0, 3, 1, 2, 4)
                .reshape(NT * 128, KG * 2 * sh)
            )
        adjt_np = np.ascontiguousarray(np.concatenate(halves, axis=0))
        in_maps.append(
            {
                "adjt": adjt_np,
                "xc": xcf,
                "w1": w1f,
                "w2": w2f,
                "wout": woutf,
                "bb": bbf,
            }
        )
    return in_maps


def kernel(x, adj, W1, W2, Wout, bout):
    from concourse import bass_utils

    x = np.asarray(x)
    adj = np.asarray(adj)
    in_maps = make_in_maps(x, adj, np.asarray(W1), np.asarray(W2),
                           np.asarray(Wout), np.asarray(bout))
    if "nc" not in _nc_cache:
        _nc_cache["nc"] = build_gat_nc()
    res = bass_utils.run_bass_kernel_spmd(
        _nc_cache["nc"], in_maps, core_ids=list(range(NCORES))
    )
    return np.concatenate([r["out"] for r in res.results], axis=0).astype(np.float32)


# revision 21
# speedup vs baseline: 1.8179x; 1.0020x over previous
"""Bass/Trainium2 kernel for the (dead-attention) GAT reference.

Effective math (see reference):
    h1  = x @ W1f                 W1f = W1.transpose(1,0,2).reshape(256,128)
    hp1 = elu(adj @ h1)
    h2  = hp1 @ W2f               W2f = W2.transpose(1,0,2).reshape(128,128)
    hp2 = elu(adj @ h2)
    y   = elu(hp2 @ Wout + bout)
    out = log_softmax(y, axis=1)

Distribution + precision strategy:
  * adj row-sharded 8 ways; each core's 2048x16384 shard is uploaded
    TRANSPOSED, fp8(e4m3), pre-tiled for big contiguous DMA lines, with
    contraction rows ordered [all ranks' first node-half | second half]
    so all layer-2 stationaries that depend on the final AllGather are
    consumed last.  Every adj tile is split across BOTH hardware DMA
    rings so a briefly-blocked ring never strands half the stream.
  * h1 is computed replicated from an fp8 x stream, one DoubleRow
    matmul per 128-node block (W1 pre-scaled by 16 to dodge fp8
    subnormals; the PSUM->SBUF cast divides it back).  The h1
    STATIONARY stays fp16: layer-1 h precision dominates final error
    (it is amplified by both all-positive adj matmuls).  h1 for tile
    g+1 is emitted ahead of tile g's adj matmuls so the PE never waits
    on the PSUM->SBUF cast.
  * Layer 2 uses fp8 h2 x fp8 adj with DoubleRow.  h2 is exchanged
    NODE-major (own blocks PE-transposed before the fp8 AllGather); the
    gathered buffer is fetched with plain static DMAs (rank-piece
    granularity for the second half) and used directly as matmul
    stationaries - no unpack transposes, no register-indexed DMAs.
  * Each layer is split into two output-column halves: the half-0 h2
    AllGather flies under layer 1's half-1 stream; layer 2 first
    consumes every first-half contraction pair of BOTH output halves
    (~35us runway) to cover the second AllGather, and half 0's output
    stage overlaps half 1's stream.
"""

import sys

import numpy as np

sys.path.insert(0, "/opt/trn_rl_repo")

N = 16384  # nodes
F = 256  # input features
D = 128  # hidden width (nheads*nhid)
C = 32  # classes
NCORES = 8
S = N // NCORES  # rows per core

_nc_cache = {}


def build_gat_nc(n_total=N, ncores=NCORES, enable_asserts=False, abufs=13):
    """Build the SPMD Bass program (one program, runs on all cores)."""
    from contextlib import ExitStack

    from concourse import bacc, bass, masks, mybir, tile

    s = n_total // ncores  # shard rows per core
    sh = s // 2  # output columns per half
    NB = n_total // 128  # contraction blocks
    NBH = NB // 2  # blocks per contraction-half
    BS = NBH // ncores  # blocks per rank segment (per half)
    assert BS % 2 == 0, "pairs must not straddle rank segments"
    KG = 4  # contraction pairs per adj tile
    NT = NB // (KG * 2)  # adj tiles per output-half phase
    NXG = NB // 8  # xc groups (8 blocks each)
    RH = sh // 128  # 128-node output chunks per half
    CW = [min(512, sh - o) for o in range(0, sh, 512)]
    CO = list(range(0, sh, 512))
    GSTAR = NBH // (KG * 2)  # first L2 tile with second-half pairs
    f32 = mybir.dt.float32
    f16 = mybir.dt.float16
    f8 = mybir.dt.float8e4
    AF = mybir.ActivationFunctionType
    OP = mybir.AluOpType
    PM = mybir.MatmulPerfMode

    nc = bacc.Bacc(
        "TRN2",
        target_bir_lowering=False,
        debug=False,
        enable_asserts=enable_asserts,
        num_devices=ncores,
    )

    adjt = nc.dram_tensor("adjt", [2 * NT * 128, KG * 2 * sh], f8, kind="ExternalInput")
    xc = nc.dram_tensor("xc", [NXG * 128, 8 * F], f8, kind="ExternalInput")
    w1 = nc.dram_tensor("w1", [F, D], f8, kind="ExternalInput")  # pre-scaled x16
    w2 = nc.dram_tensor("w2", [D, D], f16, kind="ExternalInput")
    wout = nc.dram_tensor("wout", [D, C], f32, kind="ExternalInput")
    bb = nc.dram_tensor("bb", [128, (sh // 128) * C], f32, kind="ExternalInput")
    out = nc.dram_tensor("out", [s, C], f32, kind="ExternalOutput")

    rg = [list(range(ncores))]

    with ExitStack() as stack:
        tc = stack.enter_context(tile.TileContext(nc))
        pool = lambda **kw: stack.enter_context(tc.tile_pool(**kw))
        dram = pool(name="dram", bufs=1, space="DRAM")
        const = pool(name="const", bufs=1)
        hs1p = pool(name="hs1p", bufs=1)
        hfp = pool(name="hfp", bufs=2)
        apool = pool(name="adjs", bufs=abufs)
        xcpool = pool(name="xcp", bufs=3)
        xepool = pool(name="xe", bufs=1)
        x2hp = pool(name="x2hp", bufs=2)
        h2sp = pool(name="h2sp", bufs=2)
        h2np = pool(name="h2np", bufs=2)
        tmp = pool(name="tmp", bufs=2)
        outp = pool(name="outp", bufs=4)
        stat = pool(name="stat", bufs=2)
        psb = pool(name="psb", bufs=2, space="PSUM")
        pss = pool(name="pss", bufs=3, space="PSUM")
        psy = pool(name="psy", bufs=1, space="PSUM")
        if True:
            ringA, ringB, ringC = nc.sync, nc.scalar, nc.gpsimd

            # --- replicated constants (SWDGE so HW rings start streaming) ---
            w1s = const.tile([128, 2, D], f8, tag="w1s")
            ringC.dma_start(w1s[:], w1.ap().rearrange("(a p) m -> p a m", p=128))
            w2s = const.tile([128, D], f16, tag="w2s")
            ringC.dma_start(w2s[:], w2.ap())
            wouts = const.tile([128, C], f32, tag="wouts")
            ringC.dma_start(wouts[:], wout.ap())
            bbs = const.tile([128, RH * C], f32, tag="bbs")
            ringC.dma_start(bbs[:], bb.ap())
            ident = const.tile([128, 128], f8, tag="ident")
            masks.make_identity(nc, ident[:])

            # --- DRAM bounce buffers for the two half AllGathers (fp8) ---
            h2b = [dram.tile([128, BS * D], f8, tag="h2b", name=f"h2b{h}")
                   for h in (0, 1)]
            h2f = [
                dram.tile([128 * ncores, BS * D], f8, tag="h2f", name=f"h2f{h}",
                          addr_space="Shared")
                for h in (0, 1)
            ]
            # gathered node-major h2, used directly as layer-2 stationaries
            hfull = [
                hfp.tile([128, ncores, BS, D], f8, tag="hfull", name=f"hfull{h}")
                for h in (0, 1)
            ]

            ar = adjt.ap().rearrange("(t p) m -> t p m", p=128)
            xr = xc.ap().rearrange("(g p) q -> g p q", p=128)
            h2n_t = [None, None]

            # layer-1 stationaries: even contraction pairs fp16, odd pairs
            # fp8 DoubleRow (empirically 1.2e-2 final rel err vs the 2e-2
            # gate; halves the fp16 LDW+MM cost for half the pairs)
            hs1 = hs1p.tile([128, NB // 4, 2, D], f16, tag="hs1")
            hs18 = hs1p.tile([128, NB // 4, 2, D], f8, tag="hs18")

            def stream_tile(hh2, g):
                # split every tile across BOTH rings: a stalled ring then
                # never strands half of the remaining stream
                at = apool.tile([128, KG, 2, sh], f8, tag="adj")
                t = hh2 * NT + g
                half = KG // 2
                ringA.dma_start(
                    at[:, :half, :, :].rearrange("p j i m -> p (j i m)"),
                    ar[t][:, : KG * sh],
                )
                ringB.dma_start(
                    at[:, half:, :, :].rearrange("p j i m -> p (j i m)"),
                    ar[t][:, KG * sh :],
                )
                return at

            xg_t = {}

            def load_xg(g):
                if 0 <= g < NXG:
                    t = xcpool.tile([128, 8, 2, 128], f8, tag="xg", name=f"xg{g}")
                    (ringB if g % 2 == 0 else ringA).dma_start(
                        t.rearrange("p j a m -> p (j a m)"), xr[g]
                    )
                    xg_t[g] = t

            def h1_block(k):
                # one DoubleRow matmul: both 128-feature halves in one pass
                g, b = divmod(k, 8)
                ph = pss.tile([128, D], f32, tag="pss", name=f"ph1_{k}")
                nc.tensor.matmul(
                    ph[:], xg_t[g][:, b, :, :], w1s[:], start=True, stop=True,
                    perf_mode=PM.DoubleRow,
                )
                # w1 was pre-scaled by 16; cast back on the way out of PSUM
                kp = k // 2
                dst = (hs18 if kp % 2 == 1 else hs1)[:, kp // 2, k % 2, :]
                nc.vector.tensor_scalar(
                    dst, ph[:], 1.0 / 16.0, 0.0, op0=OP.mult, op1=OP.add
                )

            def elu_half(ps, dst, off):
                # dst[:, off:off+sh] = elu(psum chunks); dst dtype may be f16
                for n, (o, w) in enumerate(zip(CO, CW)):
                    neg = tmp.tile([128, 512], f32, tag="neg", name=f"neg{n}")
                    nc.vector.tensor_scalar_min(neg[:, :w], ps[n][:], 0.0)
                    ex = tmp.tile([128, 512], f32, tag="ex", name=f"ex{n}")
                    nc.scalar.activation(ex[:, :w], neg[:, :w], AF.Exp)
                    pm1 = tmp.tile([128, 512], f32, tag="pm1", name=f"pm1{n}")
                    nc.vector.tensor_scalar(
                        pm1[:, :w], ps[n][:], 0.0, -1.0, op0=OP.max, op1=OP.add
                    )
                    nc.vector.tensor_add(
                        dst[:, off + o : off + o + w], ex[:, :w], pm1[:, :w]
                    )

            # ================= layer 1 (+ h1 pipelined one tile ahead) =====
            for hh2 in (0, 1):
                ps = [
                    psb.tile([128, w], f32, tag=f"bg{n}", name=f"ps1_{hh2}_{n}")
                    for n, w in enumerate(CW)
                ]
                if hh2 == 0:
                    load_xg(0)
                    load_xg(1)
                    for b in range(8):
                        h1_block(b)
                for g in range(NT):
                    at = stream_tile(hh2, g)
                    if hh2 == 0:
                        load_xg(g + 2)
                        if g + 1 < NXG:
                            for b in range(8):
                                h1_block((g + 1) * 8 + b)
                    for j in range(KG):
                        kp = g * KG + j
                        if kp % 2 == 1:
                            # odd pair: one fp8 DoubleRow pass for both blocks
                            for n, (o, w) in enumerate(zip(CO, CW)):
                                nc.tensor.matmul(
                                    ps[n][:],
                                    hs18[:, kp // 2, :, :],
                                    at[:, j, :, o : o + w],
                                    start=False,
                                    stop=(kp == NB // 2 - 1),
                                    perf_mode=PM.DoubleRow,
                                )
                        else:
                            for i in (0, 1):
                                k = kp * 2 + i
                                for n, (o, w) in enumerate(zip(CO, CW)):
                                    nc.tensor.matmul(
                                        ps[n][:],
                                        hs1[:, kp // 2, i, :],
                                        at[:, j, i, o : o + w],
                                        start=(k == 0),
                                        stop=False,
                                    )
                # ---- boundary hh2: elu -> f16, h2, fp8 node-major, AG ----
                x2h = x2hp.tile([128, sh], f16, tag="x2h", name=f"x2h{hh2}")
                elu_half(ps, x2h, 0)
                hT = h2sp.tile([128, sh], f8, tag="h2sT", name=f"h2sT{hh2}")
                for cb in range(RH):
                    cs = slice(cb * 128, (cb + 1) * 128)
                    ph2 = pss.tile([128, D], f32, tag="pss", name=f"ph2_{hh2}_{cb}")
                    nc.tensor.matmul(ph2[:], w2s[:], x2h[:, cs], start=True, stop=True)
                    nc.vector.tensor_copy(hT[:, cs], ph2[:])
                # transpose own blocks to node-major BEFORE the AllGather
                # (fp8 transpose writes 16-bit lanes -> step-2 output view)
                h2n = h2np.tile([128, BS, D], f8, tag="h2n", name=f"h2n{hh2}")
                h2n_t[hh2] = h2n
                for b in range(BS):
                    pt = pss.tile([128, 128, 2], f8, tag="pss", name=f"ptl_{hh2}_{b}")
                    nc.tensor.transpose(
                        pt[:, :, 0], hT[:, b * 128 : (b + 1) * 128], ident[:]
                    )
                    nc.vector.tensor_copy(h2n[:, b, :], pt[:, :, 0])
                ringC.dma_start(h2b[hh2][:], h2n.rearrange("p b d -> p (b d)"))
                nc.gpsimd.collective_compute(
                    "AllGather",
                    OP.bypass,
                    ins=[h2b[hh2].opt()],
                    outs=[h2f[hh2].opt()],
                    replica_groups=rg,
                )
            # first-half fetch on the scalar HW ring, after the whole L1
            # loop: keeping it off the gpsimd queue lets the second
            # AllGather trigger immediately at layer-1 end
            for r in range(ncores):
                ringB.dma_start(
                    hfull[0][:, r, :, :].rearrange("p b d -> p (b d)"),
                    h2f[0][r * 128 : (r + 1) * 128, :],
                )

            def lhsT_pair(kp):
                kk = 2 * kp
                hh, off = divmod(kk, NBH)
                rr, b = divmod(off, BS)
                return hfull[hh][:, rr, b : b + 2, :]

            # ================= layer 2 (fp8 DoubleRow) ====================
            x3t = xepool.tile([128, s], f32, tag="xe", name="x3t")

            def out_stage(hh2):
                outr = out.ap().rearrange("(c p) m -> c p m", p=128)
                pya = psy.tile([128, RH, C], f32, tag="psy", name=f"pya{hh2}")
                for cb in range(RH):
                    cs = slice(hh2 * sh + cb * 128, hh2 * sh + (cb + 1) * 128)
                    nc.tensor.matmul(
                        pya[:, cb, :], x3t[:, cs], wouts[:], start=True, stop=True
                    )
                zbig = outp.tile([128, RH, C], f32, tag="zbig", name=f"zbig{hh2}")
                nc.vector.tensor_add(
                    zbig.rearrange("p c m -> p (c m)"),
                    pya.rearrange("p c m -> p (c m)"),
                    bbs[:],
                )
                zf = zbig.rearrange("p c m -> p (c m)")
                negb = tmp.tile([128, RH * C], f32, tag="negB", name=f"negb{hh2}")
                nc.vector.tensor_scalar_min(negb[:], zf, 0.0)
                eb = tmp.tile([128, RH * C], f32, tag="exB", name=f"eb{hh2}")
                nc.scalar.activation(eb[:], negb[:], AF.Exp)
                pmb = tmp.tile([128, RH * C], f32, tag="pmB", name=f"pmb{hh2}")
                nc.vector.tensor_scalar(pmb[:], zf, 0.0, -1.0, op0=OP.max, op1=OP.add)
                zzb = outp.tile([128, RH, C], f32, tag="zzb", name=f"zzb{hh2}")
                nc.vector.tensor_add(zzb.rearrange("p c m -> p (c m)"), eb[:], pmb[:])
                negm = stat.tile([128, RH], f32, tag="negm", name=f"negm{hh2}")
                nc.vector.tensor_reduce(
                    negm[:], zzb[:], axis=mybir.AxisListType.X, op=OP.max, negate=True
                )
                ssum = stat.tile([128, RH], f32, tag="ssum", name=f"ssum{hh2}")
                es = tmp.tile([128, RH * C], f32, tag="negB", name=f"es{hh2}")
                esv = es.rearrange("p (c m) -> p c m", m=C)
                for cb in range(RH):
                    nc.scalar.activation(
                        esv[:, cb, :],
                        zzb[:, cb, :],
                        AF.Exp,
                        bias=negm[:, cb : cb + 1],
                        accum_out=ssum[:, cb : cb + 1],
                    )
                lse = stat.tile([128, RH], f32, tag="lse", name=f"lse{hh2}")
                nc.scalar.activation(lse[:], ssum[:], AF.Ln)
                osb = outp.tile([128, RH, C], f32, tag="osb", name=f"osb{hh2}")
                for cb in range(RH):
                    nc.vector.tensor_scalar(
                        osb[:, cb, :],
                        zzb[:, cb, :],
                        negm[:, cb : cb + 1],
                        lse[:, cb : cb + 1],
                        op0=OP.add,
                        op1=OP.subtract,
                    )
                # one batched store per half instead of RH serialized ones
                outh = out.ap().rearrange("(h c p) m -> h p c m", h=2, p=128)
                (ringA if hh2 == 0 else ringB).dma_start(outh[hh2], osb[:])

            ps2 = {
                hh2: [
                    psb.tile([128, w], f32, tag=f"bg{n}", name=f"ps2_{hh2}_{n}")
                    for n, w in enumerate(CW)
                ]
                for hh2 in (0, 1)
            }
            segs = [(0, 0, GSTAR), (1, 0, GSTAR), (0, GSTAR, NT), (1, GSTAR, NT)]
            for si, (hh2, g0, g1) in enumerate(segs):
                if si == 2:
                    # second gathered half: static rank-piece fetches on the
                    # scalar HW ring, in consumption order
                    for r in range(ncores):
                        ringB.dma_start(
                            hfull[1][:, r, :, :].rearrange("p b d -> p (b d)"),
                            h2f[1][r * 128 : (r + 1) * 128, :],
                        )
                for g in range(g0, g1):
                    at = stream_tile(hh2, g)
                    for j in range(KG):
                        kp = g * KG + j
                        for n, (o, w) in enumerate(zip(CO, CW)):
                            nc.tensor.matmul(
                                ps2[hh2][n][:],
                                lhsT_pair(kp),
                                at[:, j, :, o : o + w],
                                start=(kp == 0),
                                stop=(kp == NB // 2 - 1),
                                perf_mode=PM.DoubleRow,
                            )
                if si == 2:
                    elu_half(ps2[0], x3t, 0)
                    out_stage(0)
                elif si == 3:
                    elu_half(ps2[1], x3t, sh)
                    out_stage(1)

    nc.compile()
    return nc


def make_in_maps(x, adj, W1, W2, Wout, bout, ncores=NCORES):
    import ml_dtypes

    E8 = ml_dtypes.float8_e4m3  # TRN fp8e4 (IEEE-ish, max +-240)
    n_total = adj.shape[0]
    s = n_total // ncores
    sh = s // 2
    NB = n_total // 128
    KG = 4
    NT = NB // (KG * 2)
    NXG = NB // 8
    RH = sh // 128
    f, d = W1.shape[1], W1.shape[0] * W1.shape[2]
    w1f = np.ascontiguousarray(
        (W1.transpose(1, 0, 2).reshape(f, d) * 16.0).astype(E8)
    )
    w2f = np.ascontiguousarray(W2.transpose(1, 0, 2).reshape(d, d).astype(np.float16))
    woutf = np.ascontiguousarray(Wout.astype(np.float32))
    bbf = np.ascontiguousarray(
        np.broadcast_to(
            np.tile(bout.astype(np.float32), RH), (128, RH * Wout.shape[1])
        )
    )
    adj8 = adj.astype(E8)
    x8 = x.astype(E8)
    # global contraction order: [all ranks' half 0 | all ranks' half 1]
    perm = np.concatenate(
        [
            np.arange(r * s + hh * sh, r * s + (hh + 1) * sh)
            for hh in (0, 1)
            for r in range(ncores)
        ]
    )
    xtc = x8[perm].T  # [F, n_total]
    xcf = np.ascontiguousarray(
        xtc.reshape(2, 128, NXG, 8, 128)
        .transpose(2, 1, 3, 0, 4)
        .reshape(NXG * 128, 8 * f)
    )
    in_maps = []
    for c in range(ncores):
        rows = slice(c * s, (c + 1) * s)
        adjtc = adj8[rows][:, perm].T  # [n_total (perm), s]
        halves = []
        for hh2 in (0, 1):
            Ah = adjtc[:, hh2 * sh : (hh2 + 1) * sh]
            halves.append(
                Ah.reshape(NT, KG, 2, 128, sh)
                .transpose(0, 3, 1, 2, 4)
                .reshape(NT * 128, KG * 2 * sh)
            )
        adjt_np = np.ascontiguousarray(np.concatenate(halves, axis=0))
        in_maps.append(
            {
                "adjt": adjt_np,
                "xc": xcf,
                "w1": w1f,
                "w2": w2f,
                "wout": woutf,
                "bb": bbf,
            }
        )
    return in_maps


def kernel(x, adj, W1, W2, Wout, bout):
    from concourse import bass_utils

    x = np.asarray(x)
    adj = np.asarray(adj)
    in_maps = make_in_maps(x, adj, np.asarray(W1), np.asarray(W2),
                           np.asarray(Wout), np.asarray(bout))
    if "nc" not in _nc_cache:
        _nc_cache["nc"] = build_gat_nc()
    res = bass_utils.run_bass_kernel_spmd(
        _nc_cache["nc"], in_maps, core_ids=list(range(NCORES))
    )
    return np.concatenate([r["out"] for r in res.results], axis=0).astype(np.float32)
